# revision 1
# baseline (speedup 1.0000x reference)
"""Causal self-attention (GPT-style, B=4 T=2048 C=768 H=12) on 8 trn2 cores.

Sharding: core = (batch b, head-group g) with g in {0,1} covering 6 heads.
Each core computes qkv projections for its 6 heads, causal flash-style
attention, and a partial c_proj (its 384 contraction rows).  The pair of
cores holding the same batch produce partial sums; the host adds them
(tensor-parallel unshard) and adds b_proj.

Device dataflow (per core), fp32 storage with float32r (full-rate) matmuls:
  x^T slices (host-pretransposed, DMA'd per q-tile) -> Q^T,K^T d-major,
  V token-major with interleaved all-ones 64-col blocks.
  S^T[k,q] = K Q^T per head, two heads row-packed in the 128-deep PE array.
  P^T = exp(S^T/8) on ScalarE (PSUM->SBUF); causal triangle of diagonal
  k-tiles zeroed post-exp by a gpsimd multiply; fully-masked columns are
  skipped via restricted APs.
  [V_h | 1] single M=128 matmul accumulates y^T (64 partitions) and the
  softmax row-sums r (other 64) per (head, k-tile) into one PSUM bank.
  Normalize: evict to SBUF, gpsimd partition-shifts r opposite its y half,
  single-pass Newton reciprocal, y^T *= 1/r.
  proj: out[t,e] = sum_f y^T[f,t] wp[f,e], accumulated over head pairs.
"""

from contextlib import ExitStack

import numpy as np

import concourse.bass as bass
import concourse.mybir as mybir
import concourse.tile as tile
from concourse import bacc
from concourse.masks import make_upper_triangular

AF = mybir.ActivationFunctionType
F32 = mybir.dt.float32
F32R = mybir.dt.float32r

C = 768          # model dim
D = 64           # head dim
HG = 6           # heads per core
NP = 3           # head pairs per core
GC = HG * D      # 384 group channels
CT = C // 128    # 6 contraction tiles
QBLK = 512       # query tile (psum bank)
KBLK = 128       # key tile (partition dim)




def build_nc(T=2048):
    NQ = T // QBLK
    NK = T // KBLK
    nc = bacc.Bacc(None)

    xt_d = nc.dram_tensor("xt", [C, T], F32R, kind="ExternalInput")
    wa_d = nc.dram_tensor("wa", [C, 3 * GC], F32R, kind="ExternalInput")
    bqk_d = nc.dram_tensor("bqk", [128, 2, NP], F32, kind="ExternalInput")
    wp_d = nc.dram_tensor("wp", [GC, C], F32R, kind="ExternalInput")
    out_d = nc.dram_tensor("out", [T, C], F32, kind="ExternalOutput")

    with ExitStack() as ctx:
        tc = ctx.enter_context(tile.TileContext(nc))
        const = ctx.enter_context(tc.tile_pool(name="const", bufs=1))
        big = ctx.enter_context(tc.tile_pool(name="big", bufs=1))
        xtp = ctx.enter_context(tc.tile_pool(name="xtp", bufs=2))
        qtp = ctx.enter_context(tc.tile_pool(name="qtp", bufs=2))
        ytp = ctx.enter_context(tc.tile_pool(name="ytp", bufs=3))
        ptp = ctx.enter_context(tc.tile_pool(name="ptp", bufs=8))
        rp = ctx.enter_context(tc.tile_pool(name="rp", bufs=2))
        obp = ctx.enter_context(tc.tile_pool(name="obp", bufs=2))
        psA = ctx.enter_context(tc.tile_pool(name="psA", bufs=3, space="PSUM"))
        psY = ctx.enter_context(tc.tile_pool(name="psY", bufs=3, space="PSUM"))
        psQ = ctx.enter_context(tc.tile_pool(name="psQ", bufs=1, space="PSUM"))

        ones_f = const.tile([128, NP, D], F32)
        nc.vector.memset(ones_f, 1.0)
        # multiplicative causal mask: 1 on/above the diagonal, 0 below
        mask_sb = const.tile([128, KBLK], F32)
        make_upper_triangular(nc, mask_sb, val=1.0, diag=True)
        bqk_sb = const.tile([128, 2, NP], F32)
        nc.gpsimd.dma_start(out=bqk_sb, in_=bqk_d[:, :, :])

        wa = big.tile([128, CT, 3 * GC], F32R)
        wp = big.tile([128, NP, C], F32R)
        kt = big.tile([128, NP, T], F32R)
        # V interleaved with ones columns: even head h -> [V_h | 1],
        # odd head h -> [1 | V_h]; a single M=128 matmul then yields
        # y^T on one 64-partition half and the exp row-sums on the other.
        vs = big.tile([128, NK, HG, 2 * D], F32R)

        xt_r = xt_d[:, :].rearrange("(ct r) t -> ct r t", r=128)
        wa_r = wa_d[:, :].rearrange("(ct r) j -> ct r j", r=128)
        wp_r = wp_d[:, :].rearrange("(p r) e -> p r e", r=128)

        for q in range(NQ):
            qs = q * QBLK
            qt = qtp.tile([128, NP, QBLK], F32R, tag="qt", name="qt")
            yt = ytp.tile([128, NP, QBLK], F32R, tag="yt", name="yt")
            # x^T slice for this qtile: feeds its Q/K and its V k-range
            xtq = xtp.tile([128, CT, QBLK], F32R, tag="xtq", name="xtq")
            for ct in range(CT):
                nc.sync.dma_start(out=xtq[:, ct, :],
                                  in_=xt_r[ct][:, qs:qs + QBLK])
            if q == 0:
                # weights stream in behind the first x^T slice
                for ct in range(CT):
                    nc.sync.dma_start(out=wa[:, ct, :], in_=wa_r[ct])
            # Q^T / K^T (d-major) for this q-range, all pairs
            for p in range(NP):
                for which in (0, 1):
                    pqk = psQ.tile([128, QBLK], F32, tag="pq", name="pqk")
                    for ct in range(CT):
                        nc.tensor.matmul(
                            pqk,
                            lhsT=wa[:, ct, which * GC + p * 128:
                                           which * GC + (p + 1) * 128],
                            rhs=xtq[:, ct, :],
                            start=(ct == 0), stop=(ct == CT - 1))
                    if which == 0:
                        nc.vector.tensor_scalar_add(qt[:, p, :], pqk,
                                                    bqk_sb[:, 0, p:p + 1])
                    else:
                        nc.vector.tensor_scalar_add(kt[:, p, qs:qs + QBLK],
                                                    pqk,
                                                    bqk_sb[:, 1, p:p + 1])

            # V (+ interleaved ones) for this qtile's k-range
            for k_i in range(4 * q, 4 * (q + 1)):
                pv = psQ.tile([128, GC], F32, tag="pq", name="pv")
                for ct in range(CT):
                    kl = k_i - 4 * q
                    nc.tensor.matmul(
                        pv,
                        lhsT=xtq[:, ct, kl * KBLK:(kl + 1) * KBLK],
                        rhs=wa[:, ct, 2 * GC:3 * GC],
                        start=(ct == 0), stop=(ct == CT - 1))
                pv3 = pv.rearrange("r (a b d) -> r a b d", b=2, d=D)
                vs4 = vs[:, k_i].rearrange("r (a b) e -> r a b e", b=2)
                nc.vector.tensor_copy(vs4[:, :, 0, 0:D], pv3[:, :, 0, :])
                nc.vector.tensor_copy(vs4[:, :, 1, D:2 * D], pv3[:, :, 1, :])
                nc.vector.tensor_copy(vs4[:, :, 0, D:2 * D], ones_f)
                nc.vector.tensor_copy(vs4[:, :, 1, 0:D], ones_f)

            for p in range(NP):
                nkt = (q + 1) * (QBLK // KBLK)
                ya = psY.tile([128, QBLK], F32, tag="y", name="ya")
                yb = psY.tile([128, QBLK], F32, tag="y", name="yb")
                for k_i in range(nkt):
                    m = k_i - 4 * q
                    col0 = max(m, 0) * KBLK
                    first = (k_i == 0)
                    last = (k_i == nkt - 1)
                    for s in range(2):
                        st = psA.tile([128, QBLK], F32, tag="st", name="st")
                        pt = ptp.tile([128, QBLK], F32R, tag="pt", name="pt")
                        hoff = 64 * s
                        nc.tensor.matmul(
                            st[:, col0:QBLK],
                            lhsT=kt[hoff:hoff + 64, p,
                                       k_i * KBLK:(k_i + 1) * KBLK],
                            rhs=qt[hoff:hoff + 64, p, col0:QBLK],
                            start=True, stop=True)
                        nc.scalar.activation(pt[:, col0:QBLK],
                                             st[:, col0:QBLK],
                                             AF.Exp, scale=0.125)
                        if m >= 0:
                            seg = pt[:, col0:col0 + KBLK]
                            nc.gpsimd.tensor_mul(seg, seg, mask_sb)
                        h = 2 * p + s
                        yy = ya if s == 0 else yb
                        nc.tensor.matmul(
                            yy[:, col0:QBLK],
                            lhsT=vs[:, k_i, h, :],
                            rhs=pt[:, col0:QBLK],
                            start=first, stop=last,
                            skip_group_check=True)
                # normalize: y^T / r.  Evict PSUM fast (frees psY slots),
                # partition-shift r via gpsimd, single-pass reciprocal.
                ya_sb = rp.tile([128, QBLK], F32, tag="ya", name="ya_sb")
                yb_sb = rp.tile([128, QBLK], F32, tag="yb", name="yb_sb")
                nc.vector.tensor_copy(ya_sb, ya)
                nc.vector.tensor_copy(yb_sb, yb)
                rsh = rp.tile([128, QBLK], F32, tag="rsh", name="rsh")
                nc.gpsimd.tensor_copy(rsh[0:64, :], ya_sb[64:128, :])
                nc.gpsimd.tensor_copy(rsh[64:128, :], yb_sb[0:64, :])
                rec = rp.tile([128, QBLK], F32, tag="rec", name="rec")
                nc.vector.reciprocal_approx_fast(rec, rsh)
                nc.vector.tensor_mul(yt[0:64, p, :], ya_sb[0:64, :],
                                     rec[0:64, :])
                nc.vector.tensor_mul(yt[64:128, p, :], yb_sb[64:128, :],
                                     rec[64:128, :])

            # partial c_proj for this q-range
            if q == 0:
                for pp in range(NP):
                    nc.sync.dma_start(out=wp[:, pp, :], in_=wp_r[pp])
            for tt in range(QBLK // KBLK):
                t0 = qs + tt * KBLK
                ob = obp.tile([128, C], F32, tag="ob", name="ob")
                for ec in range(2):
                    po = psQ.tile([128, GC], F32, tag="po", name="po")
                    for j in range(NP):
                        nc.tensor.matmul(
                            po,
                            lhsT=yt[:, j, tt * KBLK:(tt + 1) * KBLK],
                            rhs=wp[:, j, ec * GC:(ec + 1) * GC],
                            start=(j == 0), stop=(j == NP - 1))
                    nc.vector.tensor_copy(ob[:, ec * GC:(ec + 1) * GC], po)
                nc.sync.dma_start(out=out_d[t0:t0 + KBLK, :], in_=ob)
    nc.compile()
    return nc


def make_in_map(x_b, w_attn, b_attn, w_proj, g):
    """Per-core input arrays for batch slice x_b and head-group g."""
    sl = slice(g * GC, (g + 1) * GC)
    wq = w_attn[:, 0 * C:1 * C][:, sl]
    wk = w_attn[:, 1 * C:2 * C][:, sl]
    wv = w_attn[:, 2 * C:3 * C][:, sl]
    bq = b_attn[0 * C:1 * C][sl]
    bk = b_attn[1 * C:2 * C][sl]
    bv = b_attn[2 * C:3 * C][sl]
    bqk = np.ascontiguousarray(
        np.stack([bq, bk]).reshape(2, NP, 128).transpose(2, 0, 1))
    return {
        "xt": np.ascontiguousarray(x_b.T),
        "wa": np.ascontiguousarray(np.concatenate([wq, wk, wv], axis=1)),
        "bqk": bqk,
        "wp": np.ascontiguousarray(w_proj[sl, :]),
    }


_NC_CACHE = {}


def _get_nc(T):
    if T not in _NC_CACHE:
        _NC_CACHE[T] = build_nc(T)
    return _NC_CACHE[T]


def kernel(x, w_attn, b_attn, w_proj, b_proj, _trace=False):
    from concourse.bass_utils import run_bass_kernel_spmd

    x = np.asarray(x, dtype=np.float32)
    w_attn = np.asarray(w_attn, dtype=np.float32)
    b_attn = np.asarray(b_attn, dtype=np.float32)
    w_proj = np.asarray(w_proj, dtype=np.float32)
    b_proj = np.asarray(b_proj, dtype=np.float32)
    B, T, _ = x.shape

    nc = _get_nc(T)
    in_maps = []
    for b in range(B):
        for g in range(2):
            in_maps.append(make_in_map(x[b], w_attn, b_attn, w_proj, g))
    res = run_bass_kernel_spmd(nc, in_maps, core_ids=list(range(2 * B)),
                               trace=_trace)
    outs = [r["out"] for r in res.results]
    # softmax rows sum to 1, so the V-bias contribution is exactly
    # bv @ w_proj added to every token (not computed on device).
    bias_row = b_proj + b_attn[2 * C:3 * C] @ w_proj
    out = np.empty((B, T, C), dtype=np.float32)
    for b in range(B):
        out[b] = outs[2 * b] + outs[2 * b + 1] + bias_row[None, :]
    if _trace:
        kernel.last_result = res
    return out



# revision 17
# speedup vs baseline: 1.1251x; 1.1251x over previous
"""Causal self-attention (GPT-style, B=4 T=2048 C=768 H=12) on 8 trn2 cores.

Sharding: core = (batch b, head-group g), g in {0,1} covering 6 heads.
Each core: qkv projections for its 6 heads, causal flash-style attention,
partial c_proj over its 384 contraction rows; host adds the two partials
per batch plus the analytic bias row.

Key device-side structure (per core):
  x^T slices (host-pretransposed) -> Q^T,K^T d-major, V token-major.
  Heads 0-3: Q/K evicted to fp8e4 in a [32, 2(d-half), T] pair layout so
  S^T = K Q^T runs as a DoubleRow matmul (half cycles/col).  The d-half
  pair layout comes free from a host-side w_attn column permutation:
  QKV psum tile A holds d0-31 of heads 0-3, tile B d32-63, so evictions
  are partition-identity copies.  Heads 4,5: bf16, plain matmul.
  exp on ScalarE over both heads of a pair at once ([128, 2, w] from a
  2-bank psum tile) -> P in bf16; causal diagonal masked post-exp by a
  gpsimd multiply.
  [V_h | 1] interleaved bf16 matmul accumulates y^T (64 partitions) and
  softmax row-sums (other 64) per (head, k-tile) into one psum bank; the
  ones blocks are memset once, V evicted with a single strided copy.
  Normalize: evict psum, gpsimd partition-shift of the row-sums, one
  fast reciprocal, two multiplies -> y^T fp32.
  proj: out[t,e] = sum_f y^T[f,t] wp[f,e] in fp32r.

Scheduling: the PE/Act/DVE streams are software-pipelined -- PV(k-1) is
issued after S(k) so the in-order PE stream never waits on exp(k); the
next q-tile's QKV/V matmul groups and the previous q-tile's c_proj are
interleaved as fillers into the attention loop so PE has work while the
Activation engine (exp) is the local bottleneck.
"""

from contextlib import ExitStack

import numpy as np

import concourse.bass as bass
import concourse.mybir as mybir
import concourse.tile as tile
from concourse import bacc
from concourse.masks import make_upper_triangular

AF = mybir.ActivationFunctionType
F32 = mybir.dt.float32
F32R = mybir.dt.float32r
BF16 = mybir.dt.bfloat16
F8 = mybir.dt.float8e4
DR = mybir.MatmulPerfMode.DoubleRow

C = 768          # model dim
D = 64           # head dim
HG = 6           # heads per core
NP = 3           # head pairs per core
GC = HG * D      # 384 group channels
CT = C // 128    # 6 contraction tiles
QBLK = 512       # query tile (psum bank)
KBLK = 128       # key tile (partition dim)

# All 6 heads run the S matmul as fp8e4 DoubleRow (half cycles/col).
# ~1.1e-2 end-to-end rel err (vs 2.2e-3 all-bf16); gate is 2e-2.
# Matmul operand base partitions are limited to {0,32,64}, so the Q/K
# channels are grouped 3 heads per 96-channel psum tile: tile t holds
# d-half (t%2) of heads 3*(t//2)..3*(t//2)+2.


def _qk_perm():
    """Channel permutation (within the 384 group channels) for Q and K.
    perm[n] = original channel feeding new channel n."""
    perm = np.empty(GC, dtype=np.int64)
    for n in range(GC):
        t, slot = divmod(n, 96)
        head = 3 * (t // 2) + slot // 32
        dd = (slot % 32) + 32 * (t % 2)
        perm[n] = head * D + dd
    return perm


def build_nc(T=2048):
    NQ = T // QBLK
    NK = T // KBLK
    nc = bacc.Bacc(None)

    xt_d = nc.dram_tensor("xt", [C, T], F32R, kind="ExternalInput")
    wa_d = nc.dram_tensor("wa", [C, 3 * GC], F32R, kind="ExternalInput")
    bqk_d = nc.dram_tensor("bqk", [128, 2, 4], F32, kind="ExternalInput")
    wp_d = nc.dram_tensor("wp", [GC, C], F32R, kind="ExternalInput")
    out_d = nc.dram_tensor("out", [T, C], F32, kind="ExternalOutput")

    with ExitStack() as ctx:
        tc = ctx.enter_context(tile.TileContext(nc))
        const = ctx.enter_context(tc.tile_pool(name="const", bufs=1))
        big = ctx.enter_context(tc.tile_pool(name="big", bufs=1))
        xtp = ctx.enter_context(tc.tile_pool(name="xtp", bufs=2))
        qtp = ctx.enter_context(tc.tile_pool(name="qtp", bufs=2))
        ytp = ctx.enter_context(tc.tile_pool(name="ytp", bufs=3))
        ptp = ctx.enter_context(tc.tile_pool(name="ptp", bufs=4))
        rp = ctx.enter_context(tc.tile_pool(name="rp", bufs=2))
        obp = ctx.enter_context(tc.tile_pool(name="obp", bufs=2))
        psS = ctx.enter_context(tc.tile_pool(name="psS", bufs=2, space="PSUM"))
        psY = ctx.enter_context(tc.tile_pool(name="psY", bufs=2, space="PSUM"))
        psB = ctx.enter_context(tc.tile_pool(name="psB", bufs=2, space="PSUM"))

        # causal mask, replicated for the two heads of an exp pair
        mask2 = const.tile([128, 2, KBLK], BF16)
        make_upper_triangular(nc, mask2[:, 0, :], val=1.0, diag=True)
        make_upper_triangular(nc, mask2[:, 1, :], val=1.0, diag=True)
        bqk_sb = const.tile([128, 2, 4], F32)
        nc.gpsimd.dma_start(out=bqk_sb, in_=bqk_d[:, :, :])

        wa = big.tile([128, CT, 3 * GC], F32R)
        wp = big.tile([128, NP, C], F32R)
        # K in fp8 d-half-pair layout: [32*(h%3)+d%32, h//3, d//32, token]
        kAB = big.tile([128, 2, 2, T], F8)
        # V interleaved with ones columns: even head h -> [V_h | 1],
        # odd head h -> [1 | V_h]; a single M=128 matmul then yields
        # y^T on one 64-partition half and the exp row-sums on the other.
        vs = big.tile([128, NK, HG, 2 * D], BF16)
        # ones blocks: within each head pair's 256 cols, the middle 128
        vs_ones = vs[:, :, :, :].rearrange("r k (a b) c -> r k a (b c)", b=2)
        nc.vector.memset(vs_ones[:, :, :, D:3 * D], 1.0)

        xt_r = xt_d[:, :].rearrange("(ct r) t -> ct r t", r=128)
        wa_r = wa_d[:, :].rearrange("(ct r) j -> ct r j", r=128)
        wp_r = wp_d[:, :].rearrange("(p r) e -> p r e", r=128)

        def dma_xtq(xtq, qs):
            for ct in range(CT):
                nc.sync.dma_start(out=xtq[:, ct, :],
                                  in_=xt_r[ct][:, qs:qs + QBLK])

        def qk_group(xtq, qAB, qs, which, t):
            """One Q-or-K 96-channel psum tile: 6 matmuls + fp8 eviction.
            Returned as a closure so callers can interleave the groups."""
            def mms():
                pqk = psB.tile([128, QBLK], F32, tag="b", name="pqk")
                for ct in range(CT):
                    nc.tensor.matmul(
                        pqk[0:96, :],
                        lhsT=wa[:, ct, which * GC + t * 96:
                                       which * GC + (t + 1) * 96],
                        rhs=xtq[:, ct, :],
                        start=(ct == 0), stop=(ct == CT - 1))
                sc = bqk_sb[0:96, which, t:t + 1]
                g, j = divmod(t, 2)
                if which == 0:
                    dest = qAB[0:96, g, j, :]
                else:
                    dest = kAB[0:96, g, j, qs:qs + QBLK]
                nc.vector.tensor_scalar_add(dest, pqk[0:96, :], sc)
            return mms

        def v_group(xtq, k_i, kl):
            def mms():
                pv = psB.tile([128, QBLK], F32, tag="b", name="pv")
                for ct in range(CT):
                    nc.tensor.matmul(
                        pv[:, 0:GC],
                        lhsT=xtq[:, ct, kl * KBLK:(kl + 1) * KBLK],
                        rhs=wa[:, ct, 2 * GC:3 * GC],
                        start=(ct == 0), stop=(ct == CT - 1))
                pv3 = pv[:, 0:GC].rearrange("r (a b d) -> r a b d", b=2, d=D)
                vsv = vs[:, k_i].rearrange("r (a b) c -> r a b c", b=2)
                # even head -> cols 0:64, odd head -> cols 64:128 of its
                # block (ones occupy the complementary halves, memset above)
                nc.vector.tensor_copy(vsv[:, :, 0, 0:D], pv3[:, :, 0, :])
                nc.vector.tensor_copy(vsv[:, :, 1, D:2 * D], pv3[:, :, 1, :])
            return mms

        def proj_group(yt, qs, tt):
            def mms():
                t0 = qs + tt * KBLK
                ob = obp.tile([128, C], F32, tag="ob", name="ob")
                for ec in range(2):
                    po = psB.tile([128, QBLK], F32, tag="b", name="po")
                    for j in range(NP):
                        nc.tensor.matmul(
                            po[:, 0:GC],
                            lhsT=yt[:, j, tt * KBLK:(tt + 1) * KBLK],
                            rhs=wp[:, j, ec * GC:(ec + 1) * GC],
                            start=(j == 0), stop=(j == NP - 1))
                    nc.vector.tensor_copy(ob[:, ec * GC:(ec + 1) * GC],
                                          po[:, 0:GC])
                    q_eng = nc.sync if ec == 0 else nc.scalar
                    q_eng.dma_start(
                        out=out_d[t0:t0 + KBLK, ec * GC:(ec + 1) * GC],
                        in_=ob[:, ec * GC:(ec + 1) * GC])
            return mms

        prev_proj = []          # proj groups of the previous q-tile
        prev_yt = None
        for q in range(NQ):
            qs = q * QBLK
            if q == 0:
                # startup: stream weights + first x^T slice, then QKV(0).
                # wa split per ct into QK/V halves so the first matmul only
                # waits on a smaller transfer; x^T of tile 1 prefetched on a
                # separate queue so attention(0) fillers aren't DMA-starved.
                xtq = xtp.tile([128, CT, QBLK], F32R, tag="xtq", name="xtq")
                for ct in range(CT):
                    nc.scalar.dma_start(out=wa[:, ct, 0:2 * GC],
                                        in_=wa_r[ct][:, 0:2 * GC])
                    nc.sync.dma_start(out=xtq[:, ct, :],
                                      in_=xt_r[ct][:, 0:QBLK])
                for ct in range(CT):
                    nc.scalar.dma_start(out=wa[:, ct, 2 * GC:3 * GC],
                                        in_=wa_r[ct][:, 2 * GC:3 * GC])
                pref_xtq = xtp.tile([128, CT, QBLK], F32R, tag="xtq",
                                    name="xtq")
                for ct in range(CT):
                    nc.gpsimd.dma_start(out=pref_xtq[:, ct, :],
                                        in_=xt_r[ct][:, QBLK:2 * QBLK])
                for pp in range(NP):
                    nc.gpsimd.dma_start(out=wp[:, pp, :], in_=wp_r[pp])
                qAB = qtp.tile([128, 2, 2, QBLK], F8, tag="qAB", name="qAB")
                for which in (0, 1):
                    for t in range(4):
                        qk_group(xtq, qAB, qs, which, t)()
                for k_i in range(4):
                    v_group(xtq, k_i, k_i)()

            # fillers to interleave into this q-tile's attention stream:
            # previous tile's c_proj, next tile's QKV + V
            fillers = list(prev_proj)
            prev_proj = []
            if q + 1 < NQ:
                nqs = qs + QBLK
                if q == 0:
                    nxtq = pref_xtq
                else:
                    nxtq = xtp.tile([128, CT, QBLK], F32R, tag="xtq",
                                    name="xtq")
                    dma_xtq(nxtq, nqs)
                nqAB = qtp.tile([128, 2, 2, QBLK], F8, tag="qAB", name="qAB")
                for which in (0, 1):
                    for t in range(4):
                        fillers.append(qk_group(nxtq, nqAB, nqs, which, t))
                for kl in range(4):
                    fillers.append(v_group(nxtq, 4 * (q + 1) + kl, kl))

            nkt = (q + 1) * (QBLK // KBLK)
            n_slots = NP * nkt
            yt = ytp.tile([128, NP, QBLK], F32R, tag="yt", name="yt")
            slot = 0
            emitted = 0
            for p in range(NP):
                ya = psY.tile([128, QBLK], F32, tag="y", name="ya")
                yb = psY.tile([128, QBLK], F32, tag="y", name="yb")
                pend = None     # software-pipelined PV of the previous k_i
                for k_i in range(nkt):
                    m = k_i - 4 * q
                    col0 = max(m, 0) * KBLK
                    st2 = psS.tile([128, 2, QBLK], F32, tag="st", name="st2")
                    pt2 = ptp.tile([128, 2, QBLK], BF16, tag="pt", name="pt2")
                    for s in range(2):
                        h = 2 * p + s
                        g, hb = divmod(h, 3)
                        base = 32 * hb
                        nc.tensor.matmul(
                            st2[:, s, col0:QBLK],
                            lhsT=kAB[base:base + 32, g, :,
                                     k_i * KBLK:(k_i + 1) * KBLK],
                            rhs=qAB[base:base + 32, g, :, col0:QBLK],
                            start=True, stop=True, perf_mode=DR)
                    nc.scalar.activation(pt2[:, :, col0:QBLK],
                                         st2[:, :, col0:QBLK],
                                         AF.Exp, scale=0.125)
                    if m >= 0:
                        seg = pt2[:, :, col0:col0 + KBLK]
                        nc.vector.tensor_mul(seg, seg, mask2)
                    if pend is not None:
                        pend()
                    first = (k_i == 0)
                    last = (k_i == nkt - 1)

                    def make_pv(pt2=pt2, p=p, col0=col0, first=first,
                                last=last, k_i=k_i, ya=ya, yb=yb):
                        def pv():
                            for s in range(2):
                                yy = ya if s == 0 else yb
                                nc.tensor.matmul(
                                    yy[:, col0:QBLK],
                                    lhsT=vs[:, k_i, 2 * p + s, :],
                                    rhs=pt2[:, s, col0:QBLK],
                                    start=first, stop=last,
                                    skip_group_check=True)
                        return pv
                    pend = make_pv()
                    # interleave cross-phase matmul groups
                    slot += 1
                    want = (slot * len(fillers)) // n_slots
                    while emitted < want:
                        fillers[emitted]()
                        emitted += 1
                pend()
                # normalize: y^T / r.  Evict psum, partition-shift r via
                # gpsimd, single-pass reciprocal, two multiplies.
                ya_sb = rp.tile([128, QBLK], F32, tag="ya", name="ya_sb")
                yb_sb = rp.tile([128, QBLK], F32, tag="yb", name="yb_sb")
                nc.vector.tensor_copy(ya_sb, ya)
                nc.vector.tensor_copy(yb_sb, yb)
                rsh = rp.tile([128, QBLK], F32, tag="rsh", name="rsh")
                nc.gpsimd.tensor_copy(rsh[0:64, :], ya_sb[64:128, :])
                nc.gpsimd.tensor_copy(rsh[64:128, :], yb_sb[0:64, :])
                rec = rp.tile([128, QBLK], F32, tag="rec", name="rec")
                nc.vector.reciprocal_approx_fast(rec, rsh)
                nc.vector.tensor_mul(yt[0:64, p, :], ya_sb[0:64, :],
                                     rec[0:64, :])
                nc.vector.tensor_mul(yt[64:128, p, :], yb_sb[64:128, :],
                                     rec[64:128, :])
            while emitted < len(fillers):
                fillers[emitted]()
                emitted += 1
            prev_proj = [proj_group(yt, qs, tt) for tt in range(QBLK // KBLK)]
            if q + 1 < NQ:
                xtq, qAB = nxtq, nqAB
        for g in prev_proj:
            g()
    nc.compile()
    return nc


def make_in_map(x_b, w_attn, b_attn, w_proj, g):
    """Per-core input arrays for batch slice x_b and head-group g."""
    sl = slice(g * GC, (g + 1) * GC)
    perm = _qk_perm()
    wq = w_attn[:, 0 * C:1 * C][:, sl][:, perm]
    wk = w_attn[:, 1 * C:2 * C][:, sl][:, perm]
    wv = w_attn[:, 2 * C:3 * C][:, sl]
    bq = b_attn[0 * C:1 * C][sl][perm]
    bk = b_attn[1 * C:2 * C][sl][perm]
    # [128, 2, 4]: per-partition bias for the 4 Q/K psum tiles (96 rows each)
    bqk = np.zeros((128, 2, 4), dtype=np.float32)
    for which, bv in enumerate((bq, bk)):
        for t in range(4):
            bqk[0:96, which, t] = bv[96 * t:96 * (t + 1)]
    return {
        "xt": np.ascontiguousarray(x_b.T),
        "wa": np.ascontiguousarray(np.concatenate([wq, wk, wv], axis=1)),
        "bqk": bqk,
        "wp": np.ascontiguousarray(w_proj[sl, :]),
    }


_NC_CACHE = {}


def _get_nc(T):
    if T not in _NC_CACHE:
        _NC_CACHE[T] = build_nc(T)
    return _NC_CACHE[T]


def kernel(x, w_attn, b_attn, w_proj, b_proj, _trace=False):
    from concourse.bass_utils import run_bass_kernel_spmd

    x = np.asarray(x, dtype=np.float32)
    w_attn = np.asarray(w_attn, dtype=np.float32)
    b_attn = np.asarray(b_attn, dtype=np.float32)
    w_proj = np.asarray(w_proj, dtype=np.float32)
    b_proj = np.asarray(b_proj, dtype=np.float32)
    B, T, _ = x.shape

    nc = _get_nc(T)
    in_maps = []
    for b in range(B):
        for g in range(2):
            in_maps.append(make_in_map(x[b], w_attn, b_attn, w_proj, g))
    res = run_bass_kernel_spmd(nc, in_maps, core_ids=list(range(2 * B)),
                               trace=_trace)
    outs = [r["out"] for r in res.results]
    # softmax rows sum to 1, so the V-bias contribution is exactly
    # bv @ w_proj added to every token (not computed on device).
    bias_row = b_proj + b_attn[2 * C:3 * C] @ w_proj
    out = np.empty((B, T, C), dtype=np.float32)
    for b in range(B):
        out[b] = outs[2 * b] + outs[2 * b + 1] + bias_row[None, :]
    if _trace:
        kernel.last_result = res
    return out


# revision 43
# speedup vs baseline: 1.2257x; 1.0894x over previous
"""Causal self-attention (GPT-style, B=4 T=2048 C=768 H=12) on 8 trn2 cores.

Sharding: core = (batch b, head-group g), g in {0,1} covering 6 heads.
Each core: qkv projections for its 6 heads, causal flash-style attention,
partial c_proj over its 384 contraction rows; host adds the two partials
per batch plus the analytic bias row.

Key device-side structure (per core):
  x^T slices (host-pretransposed) -> Q^T,K^T d-major, V token-major.
  Heads 0-3: Q/K evicted to fp8e4 in a [32, 2(d-half), T] pair layout so
  S^T = K Q^T runs as a DoubleRow matmul (half cycles/col).  The d-half
  pair layout comes free from a host-side w_attn column permutation:
  QKV psum tile A holds d0-31 of heads 0-3, tile B d32-63, so evictions
  are partition-identity copies.  Heads 4,5: bf16, plain matmul.
  exp on ScalarE over both heads of a pair at once ([128, 2, w] from a
  2-bank psum tile) -> P in bf16; causal diagonal masked post-exp by a
  gpsimd multiply.
  [V_h | 1] interleaved bf16 matmul accumulates y^T (64 partitions) and
  softmax row-sums (other 64) per (head, k-tile) into one psum bank; the
  ones blocks are memset once, V evicted with a single strided copy.
  Normalize: evict psum, gpsimd partition-shift of the row-sums, one
  fast reciprocal, two multiplies -> y^T fp32.
  proj: out[t,e] = sum_f y^T[f,t] wp[f,e] in fp32r.

Scheduling: the PE/Act/DVE streams are software-pipelined -- PV(k-1) is
issued after S(k) so the in-order PE stream never waits on exp(k); the
next q-tile's QKV/V matmul groups and the previous q-tile's c_proj are
interleaved as fillers into the attention loop so PE has work while the
Activation engine (exp) is the local bottleneck.
"""

from contextlib import ExitStack

import numpy as np

import concourse.bass as bass
import concourse.mybir as mybir
import concourse.tile as tile
from concourse import bacc
from concourse.masks import make_upper_triangular

AF = mybir.ActivationFunctionType
F32 = mybir.dt.float32
F32R = mybir.dt.float32r
BF16 = mybir.dt.bfloat16
F8 = mybir.dt.float8e4
DR = mybir.MatmulPerfMode.DoubleRow

C = 768          # model dim
D = 64           # head dim
HG = 6           # heads per core
NP = 3           # head pairs per core
GC = HG * D      # 384 group channels
CT = C // 128    # 6 contraction tiles
QBLK = 512       # query tile (psum bank)
KBLK = 128       # key tile (partition dim)

# All 6 heads run the S matmul as fp8e4 DoubleRow (half cycles/col).
# ~1.1e-2 end-to-end rel err (vs 2.2e-3 all-bf16); gate is 2e-2.
# Matmul operand base partitions are limited to {0,32,64}, so the Q/K
# channels are grouped 3 heads per 96-channel psum tile: tile t holds
# d-half (t%2) of heads 3*(t//2)..3*(t//2)+2.


def _qk_perm():
    """Channel permutation (within the 384 group channels) for Q and K.
    perm[n] = original channel feeding new channel n."""
    perm = np.empty(GC, dtype=np.int64)
    for n in range(GC):
        t, slot = divmod(n, 96)
        head = 3 * (t // 2) + slot // 32
        dd = (slot % 32) + 32 * (t % 2)
        perm[n] = head * D + dd
    return perm


_REGIONS = []      # (label, next_instruction_index) probes for trace analysis


def _mark(nc, label):
    _REGIONS.append((label,
                     int(nc.get_next_instruction_name().split("-")[-1])))


def build_nc(T=2048):
    NQ = T // QBLK
    NK = T // KBLK
    nc = bacc.Bacc(None)

    xt_d = nc.dram_tensor("xt", [C, T], BF16, kind="ExternalInput")
    wa_d = nc.dram_tensor("wa", [C, 3 * GC], BF16, kind="ExternalInput")
    bqk_d = nc.dram_tensor("bqk", [128, 2, 4], F32, kind="ExternalInput")
    wp_d = nc.dram_tensor("wp", [GC, C], F32R, kind="ExternalInput")
    out_d = nc.dram_tensor("out", [T, C], BF16, kind="ExternalOutput")

    with ExitStack() as ctx:
        tc = ctx.enter_context(tile.TileContext(nc))
        const = ctx.enter_context(tc.tile_pool(name="const", bufs=1))
        big = ctx.enter_context(tc.tile_pool(name="big", bufs=1))
        xtp = ctx.enter_context(tc.tile_pool(name="xtp", bufs=2))
        qtp = ctx.enter_context(tc.tile_pool(name="qtp", bufs=2))
        ytp = ctx.enter_context(tc.tile_pool(name="ytp", bufs=3))
        ptp = ctx.enter_context(tc.tile_pool(name="ptp", bufs=4))
        rp = ctx.enter_context(tc.tile_pool(name="rp", bufs=2))
        obp = ctx.enter_context(tc.tile_pool(name="obp", bufs=2))
        psS = ctx.enter_context(tc.tile_pool(name="psS", bufs=2, space="PSUM"))
        psY = ctx.enter_context(tc.tile_pool(name="psY", bufs=2, space="PSUM"))
        psB = ctx.enter_context(tc.tile_pool(name="psB", bufs=2, space="PSUM"))

        # causal mask, replicated for the two heads of an exp pair
        mask2 = const.tile([128, 2, KBLK], BF16)
        make_upper_triangular(nc, mask2[:, 0, :], val=1.0, diag=True)
        make_upper_triangular(nc, mask2[:, 1, :], val=1.0, diag=True)
        bqk_sb = const.tile([128, 2, 4], F32)

        wa = big.tile([128, CT, 3 * GC], BF16)
        wp = big.tile([128, NP, C], F32R)
        # K in fp8 d-half-pair layout: [32*(h%3)+d%32, h//3, d//32, token]
        kAB = big.tile([128, 2, 2, T], F8)
        # V interleaved with ones columns: even head h -> [V_h | 1],
        # odd head h -> [1 | V_h]; a single M=128 matmul then yields
        # y^T on one 64-partition half and the exp row-sums on the other.
        vs = big.tile([128, NK, HG, 2 * D], BF16)

        xt_r = xt_d[:, :].rearrange("(ct r) t -> ct r t", r=128)
        wa_r = wa_d[:, :].rearrange("(ct r) j -> ct r j", r=128)
        wp_r = wp_d[:, :].rearrange("(p r) e -> p r e", r=128)

        def dma_xtq(xtq, qs):
            for ct in range(CT):
                nc.sync.dma_start(out=xtq[:, ct, :],
                                  in_=xt_r[ct][:, qs:qs + QBLK])

        def qk_group(xtq, qAB, qs, which, t):
            """One Q-or-K 96-channel psum tile: 6 matmuls + fp8 eviction.
            Returned as a closure so callers can interleave the groups."""
            def mms():
                _mark(nc, f"qk.w{which}.t{t}")
                pqk = psB.tile([128, QBLK], F32, tag="b", name="pqk")
                for ct in range(CT):
                    nc.tensor.matmul(
                        pqk[0:96, :],
                        lhsT=wa[:, ct, which * GC + t * 96:
                                       which * GC + (t + 1) * 96],
                        rhs=xtq[:, ct, :],
                        start=(ct == 0), stop=(ct == CT - 1))
                sc = bqk_sb[0:96, which, t:t + 1]
                g, j = divmod(t, 2)
                if which == 0:
                    dest = qAB[0:96, g, j, :]
                else:
                    dest = kAB[0:96, g, j, qs:qs + QBLK]
                nc.vector.tensor_scalar_add(dest, pqk[0:96, :], sc)
            return mms

        def v_group(xtq, k_i, kl):
            def mms():
                _mark(nc, f"v.k{k_i}")
                pv = psB.tile([128, QBLK], F32, tag="b", name="pv")
                for ct in range(CT):
                    nc.tensor.matmul(
                        pv[:, 0:GC],
                        lhsT=xtq[:, ct, kl * KBLK:(kl + 1) * KBLK],
                        rhs=wa[:, ct, 2 * GC:3 * GC],
                        start=(ct == 0), stop=(ct == CT - 1))
                pv3 = pv[:, 0:GC].rearrange("r (a b d) -> r a b d", b=2, d=D)
                vsv = vs[:, k_i].rearrange("r (a b) c -> r a b c", b=2)
                vso = vs[:, k_i].rearrange("r (a b) c -> r a (b c)", b=2)
                # ones occupy the middle 128 cols of each pair's 256 block;
                # even head V -> cols 0:64, odd head V -> cols 64:128 of its
                # own block
                nc.gpsimd.memset(vso[:, :, D:3 * D], 1.0)
                nc.vector.tensor_copy(vsv[:, :, 0, 0:D], pv3[:, :, 0, :])
                nc.vector.tensor_copy(vsv[:, :, 1, D:2 * D], pv3[:, :, 1, :])
            return mms

        def proj_group(yt, qs, tt):
            def mms():
                _mark(nc, f"proj.tt{tt}")
                t0 = qs + tt * KBLK
                ob = obp.tile([128, C], BF16, tag="ob", name="ob")
                for ec in range(2):
                    po = psB.tile([128, QBLK], F32, tag="b", name="po")
                    for j in range(NP):
                        nc.tensor.matmul(
                            po[:, 0:GC],
                            lhsT=yt[:, j, tt * KBLK:(tt + 1) * KBLK],
                            rhs=wp[:, j, ec * GC:(ec + 1) * GC],
                            start=(j == 0), stop=(j == NP - 1))
                    nc.vector.tensor_copy(ob[:, ec * GC:(ec + 1) * GC],
                                           po[:, 0:GC])
                q_eng = nc.sync if tt % 2 == 0 else nc.scalar
                q_eng.dma_start(out=out_d[t0:t0 + KBLK, :], in_=ob)
            return mms

        proj_queue = []         # deferred c_proj groups of earlier q-tiles
        carry = []              # fillers deferred to the next tile
        for q in range(NQ):
            qs = q * QBLK
            if q == 0:
                # startup: the QKV matmul for contraction tile ct needs the
                # (wa-qk[ct], xtq[ct]) pair, so stream those as interleaved
                # pairs on the two HWDGE queues; everything else follows.
                xtq = xtp.tile([128, CT, QBLK], BF16, tag="xtq", name="xtq")
                for ct in range(CT):
                    nc.scalar.dma_start(out=wa[:, ct, 0:2 * GC],
                                        in_=wa_r[ct][:, 0:2 * GC])
                    nc.sync.dma_start(out=xtq[:, ct, :],
                                      in_=xt_r[ct][:, 0:QBLK])
                    if ct == 0:
                        nc.sync.dma_start(out=bqk_sb, in_=bqk_d[:, :, :])
                for ct in range(CT):
                    nc.scalar.dma_start(out=wa[:, ct, 2 * GC:3 * GC],
                                        in_=wa_r[ct][:, 2 * GC:3 * GC])
                pref_xtq = xtp.tile([128, CT, QBLK], BF16, tag="xtq",
                                    name="xtq")
                for ct in range(CT):
                    nc.sync.dma_start(out=pref_xtq[:, ct, :],
                                      in_=xt_r[ct][:, QBLK:2 * QBLK])
                for pp in range(NP):
                    nc.scalar.dma_start(out=wp[:, pp, :], in_=wp_r[pp])
                qAB = qtp.tile([128, 2, 2, QBLK], F8, tag="qAB", name="qAB")
                for which in (0, 1):
                    for t in range(4):
                        qk_group(xtq, qAB, qs, which, t)()
                for k_i in range(4):
                    v_group(xtq, k_i, k_i)()

            # fillers interleaved into this q-tile's attention stream.  The
            # early tiles are PE-rich (small attention) and the late ones
            # Act-bound, so PE-side work is skewed late: the last tile gets
            # the K-projection of its own QKV plus two tiles' worth of
            # c_proj from the deferred queue.
            fillers = list(carry)
            carry = []
            if q + 1 < NQ:
                nqs = qs + QBLK
                if q == 0:
                    nxtq = pref_xtq
                else:
                    nxtq = xtp.tile([128, CT, QBLK], BF16, tag="xtq",
                                    name="xtq")
                    dma_xtq(nxtq, nqs)
                nqAB = qtp.tile([128, 2, 2, QBLK], F8, tag="qAB", name="qAB")
                for t in range(4):
                    fillers.append(qk_group(nxtq, nqAB, nqs, 0, t))
                kq = [qk_group(nxtq, nqAB, nqs, 1, t) for t in range(4)]
                if q + 1 == NQ - 1:
                    carry += kq      # K of the last tile: emit during it
                else:
                    fillers += kq
                for kl in range(4):
                    fillers.append(v_group(nxtq, 4 * (q + 1) + kl, kl))
            # deferred c_proj: none before att(2); proj(0) at att(2);
            # the rest at att(3)
            if q == NQ - 2:
                fillers += proj_queue[:4]
                del proj_queue[:4]
            elif q == NQ - 1:
                fillers += proj_queue
                proj_queue = []

            nkt = (q + 1) * (QBLK // KBLK)
            n_slots = NP * nkt
            yt = ytp.tile([128, NP, QBLK], F32R, tag="yt", name="yt")
            slot = 0
            emitted = 0
            for p in range(NP):
                ya = psY.tile([128, QBLK], F32, tag="y", name="ya")
                yb = psY.tile([128, QBLK], F32, tag="y", name="yb")
                pend = None     # software-pipelined PV of the previous k_i
                for k_i in range(nkt):
                    _mark(nc, f"att.q{q}.p{p}.k{k_i}")
                    m = k_i - 4 * q
                    col0 = max(m, 0) * KBLK
                    st2 = psS.tile([128, 2, QBLK], F32, tag="st", name="st2")
                    pt2 = ptp.tile([128, 2, QBLK], BF16, tag="pt", name="pt2")
                    for s in range(2):
                        h = 2 * p + s
                        g, hb = divmod(h, 3)
                        base = 32 * hb
                        nc.tensor.matmul(
                            st2[:, s, col0:QBLK],
                            lhsT=kAB[base:base + 32, g, :,
                                     k_i * KBLK:(k_i + 1) * KBLK],
                            rhs=qAB[base:base + 32, g, :, col0:QBLK],
                            start=True, stop=True, perf_mode=DR)
                    nc.scalar.activation(pt2[:, :, col0:QBLK],
                                         st2[:, :, col0:QBLK],
                                         AF.Exp, scale=0.125)
                    if m >= 0:
                        seg = pt2[:, :, col0:col0 + KBLK]
                        nc.vector.tensor_mul(seg, seg, mask2)
                    if pend is not None:
                        pend()
                    first = (k_i == 0)
                    last = (k_i == nkt - 1)

                    def make_pv(pt2=pt2, p=p, col0=col0, first=first,
                                last=last, k_i=k_i, ya=ya, yb=yb):
                        def pv():
                            for s in range(2):
                                yy = ya if s == 0 else yb
                                nc.tensor.matmul(
                                    yy[:, col0:QBLK],
                                    lhsT=vs[:, k_i, 2 * p + s, :],
                                    rhs=pt2[:, s, col0:QBLK],
                                    start=first, stop=last,
                                    skip_group_check=True)
                        return pv
                    pend = make_pv()
                    # interleave cross-phase matmul groups (front-loaded so
                    # dependency-critical groups land before their readers)
                    slot += 1
                    want = min(len(fillers),
                               (3 * slot * len(fillers)) // (2 * n_slots))
                    while emitted < want:
                        fillers[emitted]()
                        emitted += 1
                pend()
                _mark(nc, f"norm.q{q}.p{p}")
                # normalize: y^T / r.  Evict psum, partition-shift r via
                # gpsimd, single-pass reciprocal, two multiplies.  For the
                # final tile's last pair this is the tail critical path, so
                # run it in 128-column slices that unblock proj(tt) early.
                ya_sb = rp.tile([128, QBLK], F32, tag="ya", name="ya_sb")
                yb_sb = rp.tile([128, QBLK], F32, tag="yb", name="yb_sb")
                rsh = rp.tile([128, QBLK], F32, tag="rsh", name="rsh")
                rec = rp.tile([128, QBLK], F32, tag="rec", name="rec")
                n_sl = 4 if (q == NQ - 1 and p == NP - 1) else 1
                for sl in range(n_sl):
                    cs = slice(sl * (QBLK // n_sl), (sl + 1) * (QBLK // n_sl))
                    nc.vector.tensor_copy(ya_sb[:, cs], ya[:, cs])
                    nc.vector.tensor_copy(yb_sb[:, cs], yb[:, cs])
                    nc.gpsimd.tensor_copy(rsh[0:64, cs], ya_sb[64:128, cs])
                    nc.gpsimd.tensor_copy(rsh[64:128, cs], yb_sb[0:64, cs])
                    nc.vector.reciprocal_approx_fast(rec[:, cs], rsh[:, cs])
                    nc.vector.tensor_mul(yt[0:64, p, cs], ya_sb[0:64, cs],
                                         rec[0:64, cs])
                    nc.vector.tensor_mul(yt[64:128, p, cs],
                                         yb_sb[64:128, cs],
                                         rec[64:128, cs])
            while emitted < len(fillers):
                fillers[emitted]()
                emitted += 1
            proj_queue += [proj_group(yt, qs, tt)
                           for tt in range(QBLK // KBLK)]
            if q + 1 < NQ:
                xtq, qAB = nxtq, nqAB
        for g in proj_queue:
            g()
    nc.compile()
    return nc


def make_in_map(x_b, w_attn, b_attn, w_proj, g):
    """Per-core input arrays for batch slice x_b and head-group g."""
    sl = slice(g * GC, (g + 1) * GC)
    perm = _qk_perm()
    wq = w_attn[:, 0 * C:1 * C][:, sl][:, perm]
    wk = w_attn[:, 1 * C:2 * C][:, sl][:, perm]
    wv = w_attn[:, 2 * C:3 * C][:, sl]
    bq = b_attn[0 * C:1 * C][sl][perm]
    bk = b_attn[1 * C:2 * C][sl][perm]
    # [128, 2, 4]: per-partition bias for the 4 Q/K psum tiles (96 rows each)
    bqk = np.zeros((128, 2, 4), dtype=np.float32)
    for which, bv in enumerate((bq, bk)):
        for t in range(4):
            bqk[0:96, which, t] = bv[96 * t:96 * (t + 1)]
    import ml_dtypes
    return {
        "xt": np.ascontiguousarray(x_b.T).astype(ml_dtypes.bfloat16),
        "wa": np.ascontiguousarray(
            np.concatenate([wq, wk, wv], axis=1)).astype(ml_dtypes.bfloat16),
        "bqk": bqk,
        "wp": np.ascontiguousarray(w_proj[sl, :]),
    }


_NC_CACHE = {}


def _get_nc(T):
    if T not in _NC_CACHE:
        _NC_CACHE[T] = build_nc(T)
    return _NC_CACHE[T]


def kernel(x, w_attn, b_attn, w_proj, b_proj, _trace=False):
    from concourse.bass_utils import run_bass_kernel_spmd

    x = np.asarray(x, dtype=np.float32)
    w_attn = np.asarray(w_attn, dtype=np.float32)
    b_attn = np.asarray(b_attn, dtype=np.float32)
    w_proj = np.asarray(w_proj, dtype=np.float32)
    b_proj = np.asarray(b_proj, dtype=np.float32)
    B, T, _ = x.shape

    nc = _get_nc(T)
    in_maps = []
    for b in range(B):
        for g in range(2):
            in_maps.append(make_in_map(x[b], w_attn, b_attn, w_proj, g))
    res = run_bass_kernel_spmd(nc, in_maps, core_ids=list(range(2 * B)),
                               trace=_trace)
    outs = [np.asarray(r["out"], dtype=np.float32) for r in res.results]
    # softmax rows sum to 1, so the V-bias contribution is exactly
    # bv @ w_proj added to every token (not computed on device).
    bias_row = b_proj + b_attn[2 * C:3 * C] @ w_proj
    out = np.empty((B, T, C), dtype=np.float32)
    for b in range(B):
        out[b] = outs[2 * b] + outs[2 * b + 1] + bias_row[None, :]
    if _trace:
        kernel.last_result = res
    return out


# revision 49
# speedup vs baseline: 1.2378x; 1.0099x over previous
"""Causal self-attention (GPT-style, B=4 T=2048 C=768 H=12) on 8 trn2 cores.

Sharding: core = (batch b, head-group g), g in {0,1} covering 6 heads.
Each core: qkv projections for its 6 heads, causal flash-style attention,
partial c_proj over its 384 contraction rows; host adds the two partials
per batch plus the analytic bias row.

Key device-side structure (per core):
  x^T slices (host-pretransposed) -> Q^T,K^T d-major, V token-major.
  Heads 0-3: Q/K evicted to fp8e4 in a [32, 2(d-half), T] pair layout so
  S^T = K Q^T runs as a DoubleRow matmul (half cycles/col).  The d-half
  pair layout comes free from a host-side w_attn column permutation:
  QKV psum tile A holds d0-31 of heads 0-3, tile B d32-63, so evictions
  are partition-identity copies.  Heads 4,5: bf16, plain matmul.
  exp on ScalarE over both heads of a pair at once ([128, 2, w] from a
  2-bank psum tile) -> P in bf16; causal diagonal masked post-exp by a
  gpsimd multiply.
  [V_h | 1] interleaved bf16 matmul accumulates y^T (64 partitions) and
  softmax row-sums (other 64) per (head, k-tile) into one psum bank; the
  ones blocks are memset once, V evicted with a single strided copy.
  Normalize: evict psum, gpsimd partition-shift of the row-sums, one
  fast reciprocal, two multiplies -> y^T fp32.
  proj: out[t,e] = sum_f y^T[f,t] wp[f,e] in fp32r.

Scheduling: the PE/Act/DVE streams are software-pipelined -- PV(k-1) is
issued after S(k) so the in-order PE stream never waits on exp(k); the
next q-tile's QKV/V matmul groups and the previous q-tile's c_proj are
interleaved as fillers into the attention loop so PE has work while the
Activation engine (exp) is the local bottleneck.
"""

from contextlib import ExitStack

import numpy as np

import concourse.bass as bass
import concourse.mybir as mybir
import concourse.tile as tile
from concourse import bacc
from concourse.masks import make_upper_triangular

AF = mybir.ActivationFunctionType
F32 = mybir.dt.float32
F32R = mybir.dt.float32r
BF16 = mybir.dt.bfloat16
F8 = mybir.dt.float8e4
DR = mybir.MatmulPerfMode.DoubleRow

C = 768          # model dim
D = 64           # head dim
HG = 6           # heads per core
NP = 3           # head pairs per core
GC = HG * D      # 384 group channels
CT = C // 128    # 6 contraction tiles
QBLK = 512       # query tile (psum bank)
KBLK = 128       # key tile (partition dim)

# All 6 heads run the S matmul as fp8e4 DoubleRow (half cycles/col).
# ~1.1e-2 end-to-end rel err (vs 2.2e-3 all-bf16); gate is 2e-2.
# Matmul operand base partitions are limited to {0,32,64}, so the Q/K
# channels are grouped 3 heads per 96-channel psum tile: tile t holds
# d-half (t%2) of heads 3*(t//2)..3*(t//2)+2.


def _qk_perm():
    """Channel permutation (within the 384 group channels) for Q and K.
    perm[n] = original channel feeding new channel n."""
    perm = np.empty(GC, dtype=np.int64)
    for n in range(GC):
        t, slot = divmod(n, 96)
        head = 3 * (t // 2) + slot // 32
        dd = (slot % 32) + 32 * (t % 2)
        perm[n] = head * D + dd
    return perm


_REGIONS = []      # (label, next_instruction_index) probes for trace analysis


def _mark(nc, label):
    _REGIONS.append((label,
                     int(nc.get_next_instruction_name().split("-")[-1])))


def build_nc(T=2048):
    NQ = T // QBLK
    NK = T // KBLK
    nc = bacc.Bacc(None)

    xt_d = nc.dram_tensor("xt", [C, T], BF16, kind="ExternalInput")
    wa_d = nc.dram_tensor("wa", [C, 3 * GC], BF16, kind="ExternalInput")
    bqk_d = nc.dram_tensor("bqk", [128, 2, 4], F32, kind="ExternalInput")
    wp_d = nc.dram_tensor("wp", [GC, C], F32R, kind="ExternalInput")
    out_d = nc.dram_tensor("out", [T, C], BF16, kind="ExternalOutput")

    with ExitStack() as ctx:
        tc = ctx.enter_context(tile.TileContext(nc))
        const = ctx.enter_context(tc.tile_pool(name="const", bufs=1))
        big = ctx.enter_context(tc.tile_pool(name="big", bufs=1))
        xtp = ctx.enter_context(tc.tile_pool(name="xtp", bufs=2))
        qtp = ctx.enter_context(tc.tile_pool(name="qtp", bufs=2))
        ytp = ctx.enter_context(tc.tile_pool(name="ytp", bufs=3))
        ptp = ctx.enter_context(tc.tile_pool(name="ptp", bufs=6))
        rp = ctx.enter_context(tc.tile_pool(name="rp", bufs=2))
        obp = ctx.enter_context(tc.tile_pool(name="obp", bufs=2))
        psS = ctx.enter_context(tc.tile_pool(name="psS", bufs=2, space="PSUM"))
        psY = ctx.enter_context(tc.tile_pool(name="psY", bufs=2, space="PSUM"))
        psB = ctx.enter_context(tc.tile_pool(name="psB", bufs=2, space="PSUM"))

        # causal mask, replicated for the two heads of an exp pair
        # (built after the startup DMAs are issued -- see q == 0 below --
        # so the gpsimd queue isn't busy ahead of the SWDGE x fetch)
        mask2 = const.tile([128, 2, KBLK], BF16)
        bqk_sb = const.tile([128, 2, 4], F32)

        wa = big.tile([128, CT, 3 * GC], BF16)
        wp = big.tile([128, NP, C], F32R)
        # K in fp8 d-half-pair layout: [32*(h%3)+d%32, h//3, d//32, token]
        kAB = big.tile([128, 2, 2, T], F8)
        # V interleaved with ones columns: even head h -> [V_h | 1],
        # odd head h -> [1 | V_h]; a single M=128 matmul then yields
        # y^T on one 64-partition half and the exp row-sums on the other.
        vs = big.tile([128, NK, HG, 2 * D], BF16)

        xt_r = xt_d[:, :].rearrange("(ct r) t -> ct r t", r=128)
        wa_r = wa_d[:, :].rearrange("(ct r) j -> ct r j", r=128)
        wp_r = wp_d[:, :].rearrange("(p r) e -> p r e", r=128)

        def dma_xtq(xtq, qs):
            for ct in range(CT):
                nc.sync.dma_start(out=xtq[:, ct, :],
                                  in_=xt_r[ct][:, qs:qs + QBLK])

        def qk_group(xtq, qAB, qs, which, t):
            """One Q-or-K 96-channel psum tile: 6 matmuls + fp8 eviction.
            Returned as a closure so callers can interleave the groups."""
            def mms():
                _mark(nc, f"qk.w{which}.t{t}")
                pqk = psB.tile([128, QBLK], F32, tag="b", name="pqk")
                for ct in range(CT):
                    nc.tensor.matmul(
                        pqk[0:96, :],
                        lhsT=wa[:, ct, which * GC + t * 96:
                                       which * GC + (t + 1) * 96],
                        rhs=xtq[:, ct, :],
                        start=(ct == 0), stop=(ct == CT - 1))
                sc = bqk_sb[0:96, which, t:t + 1]
                g, j = divmod(t, 2)
                if which == 0:
                    dest = qAB[0:96, g, j, :]
                else:
                    dest = kAB[0:96, g, j, qs:qs + QBLK]
                nc.vector.tensor_scalar_add(dest, pqk[0:96, :], sc)
            return mms

        def v_group(xtq, k_i, kl):
            def mms():
                _mark(nc, f"v.k{k_i}")
                pv = psB.tile([128, QBLK], F32, tag="b", name="pv")
                for ct in range(CT):
                    nc.tensor.matmul(
                        pv[:, 0:GC],
                        lhsT=xtq[:, ct, kl * KBLK:(kl + 1) * KBLK],
                        rhs=wa[:, ct, 2 * GC:3 * GC],
                        start=(ct == 0), stop=(ct == CT - 1))
                pv3 = pv[:, 0:GC].rearrange("r (a b d) -> r a b d", b=2, d=D)
                vsv = vs[:, k_i].rearrange("r (a b) c -> r a b c", b=2)
                vso = vs[:, k_i].rearrange("r (a b) c -> r a (b c)", b=2)
                # ones occupy the middle 128 cols of each pair's 256 block;
                # even head V -> cols 0:64, odd head V -> cols 64:128 of its
                # own block
                nc.gpsimd.memset(vso[:, :, D:3 * D], 1.0)
                nc.vector.tensor_copy(vsv[:, :, 0, 0:D], pv3[:, :, 0, :])
                nc.vector.tensor_copy(vsv[:, :, 1, D:2 * D], pv3[:, :, 1, :])
            return mms

        def proj_group(yt, qs, tt):
            def mms():
                _mark(nc, f"proj.tt{tt}")
                t0 = qs + tt * KBLK
                ob = obp.tile([128, C], BF16, tag="ob", name="ob")
                for ec in range(2):
                    po = psB.tile([128, QBLK], F32, tag="b", name="po")
                    for j in range(NP):
                        nc.tensor.matmul(
                            po[:, 0:GC],
                            lhsT=yt[:, j, tt * KBLK:(tt + 1) * KBLK],
                            rhs=wp[:, j, ec * GC:(ec + 1) * GC],
                            start=(j == 0), stop=(j == NP - 1))
                    nc.vector.tensor_copy(ob[:, ec * GC:(ec + 1) * GC],
                                           po[:, 0:GC])
                q_eng = nc.sync if tt % 2 == 0 else nc.scalar
                q_eng.dma_start(out=out_d[t0:t0 + KBLK, :], in_=ob)
            return mms

        proj_queue = []         # deferred c_proj groups of earlier q-tiles
        carry = []              # fillers deferred to the next tile
        for q in range(NQ):
            qs = q * QBLK
            if q == 0:
                # startup: the QKV matmul for contraction tile ct needs the
                # (wa-qk[ct], xtq[ct]) pair, so stream those as interleaved
                # pairs on the two HWDGE queues; everything else follows.
                xtq = xtp.tile([128, CT, QBLK], BF16, tag="xtq", name="xtq")
                for ct in range(CT):
                    nc.scalar.dma_start(out=wa[:, ct, 0:2 * GC],
                                        in_=wa_r[ct][:, 0:2 * GC])
                    nc.gpsimd.dma_start(out=xtq[:, ct, :],
                                        in_=xt_r[ct][:, 0:QBLK])
                    if ct == 0:
                        nc.sync.dma_start(out=bqk_sb, in_=bqk_d[:, :, :])
                for ct in range(CT):
                    nc.scalar.dma_start(out=wa[:, ct, 2 * GC:3 * GC],
                                        in_=wa_r[ct][:, 2 * GC:3 * GC])
                pref_xtq = xtp.tile([128, CT, QBLK], BF16, tag="xtq",
                                    name="xtq")
                for ct in range(CT):
                    nc.gpsimd.dma_start(out=pref_xtq[:, ct, :],
                                        in_=xt_r[ct][:, QBLK:2 * QBLK])
                for pp in range(NP):
                    nc.scalar.dma_start(out=wp[:, pp, :], in_=wp_r[pp])
                make_upper_triangular(nc, mask2[:, 0, :], val=1.0, diag=True)
                make_upper_triangular(nc, mask2[:, 1, :], val=1.0, diag=True)
                qAB = qtp.tile([128, 2, 2, QBLK], F8, tag="qAB", name="qAB")
                for which in (0, 1):
                    for t in range(4):
                        qk_group(xtq, qAB, qs, which, t)()
                for k_i in range(4):
                    v_group(xtq, k_i, k_i)()

            # fillers interleaved into this q-tile's attention stream.  The
            # early tiles are PE-rich (small attention) and the late ones
            # Act-bound, so PE-side work is skewed late: the last tile gets
            # the K-projection of its own QKV plus two tiles' worth of
            # c_proj from the deferred queue.
            fillers = list(carry)
            carry = []
            if q + 1 < NQ:
                nqs = qs + QBLK
                if q == 0:
                    nxtq = pref_xtq
                else:
                    nxtq = xtp.tile([128, CT, QBLK], BF16, tag="xtq",
                                    name="xtq")
                    dma_xtq(nxtq, nqs)
                nqAB = qtp.tile([128, 2, 2, QBLK], F8, tag="qAB", name="qAB")
                for t in range(4):
                    fillers.append(qk_group(nxtq, nqAB, nqs, 0, t))
                kq = [qk_group(nxtq, nqAB, nqs, 1, t) for t in range(4)]
                if q + 1 == NQ - 1:
                    carry += kq      # K of the last tile: emit during it
                else:
                    fillers += kq
                for kl in range(4):
                    fillers.append(v_group(nxtq, 4 * (q + 1) + kl, kl))
            # deferred c_proj: none before att(2); proj(0) at att(2);
            # the rest at att(3)
            if q == NQ - 1:
                fillers += proj_queue
                proj_queue = []

            nkt = (q + 1) * (QBLK // KBLK)
            n_slots = NP * nkt
            yt = ytp.tile([128, NP, QBLK], F32R, tag="yt", name="yt")
            slot = 0
            emitted = 0
            for p in range(NP):
                ya = psY.tile([128, QBLK], F32, tag="y", name="ya")
                yb = psY.tile([128, QBLK], F32, tag="y", name="yb")
                pend = None     # software-pipelined PV of the previous k_i
                for k_i in range(nkt):
                    _mark(nc, f"att.q{q}.p{p}.k{k_i}")
                    m = k_i - 4 * q
                    col0 = max(m, 0) * KBLK
                    st2 = psS.tile([128, 2, QBLK], F32, tag="st", name="st2")
                    pt2 = ptp.tile([128, 2, QBLK], BF16, tag="pt", name="pt2")
                    for s in range(2):
                        h = 2 * p + s
                        g, hb = divmod(h, 3)
                        base = 32 * hb
                        nc.tensor.matmul(
                            st2[:, s, col0:QBLK],
                            lhsT=kAB[base:base + 32, g, :,
                                     k_i * KBLK:(k_i + 1) * KBLK],
                            rhs=qAB[base:base + 32, g, :, col0:QBLK],
                            start=True, stop=True, perf_mode=DR)
                    nc.scalar.activation(pt2[:, :, col0:QBLK],
                                         st2[:, :, col0:QBLK],
                                         AF.Exp, scale=0.125)
                    if m >= 0:
                        seg = pt2[:, :, col0:col0 + KBLK]
                        nc.vector.tensor_mul(seg, seg, mask2)
                    if pend is not None:
                        pend()
                    first = (k_i == 0)
                    last = (k_i == nkt - 1)

                    def make_pv(pt2=pt2, p=p, col0=col0, first=first,
                                last=last, k_i=k_i, ya=ya, yb=yb):
                        def pv():
                            for s in range(2):
                                yy = ya if s == 0 else yb
                                nc.tensor.matmul(
                                    yy[:, col0:QBLK],
                                    lhsT=vs[:, k_i, 2 * p + s, :],
                                    rhs=pt2[:, s, col0:QBLK],
                                    start=first, stop=last,
                                    skip_group_check=True)
                        return pv
                    pend = make_pv()
                    # interleave cross-phase matmul groups (front-loaded so
                    # dependency-critical groups land before their readers)
                    slot += 1
                    want = min(len(fillers),
                               (3 * slot * len(fillers)) // (2 * n_slots))
                    while emitted < want:
                        fillers[emitted]()
                        emitted += 1
                pend()
                _mark(nc, f"norm.q{q}.p{p}")
                # normalize: y^T / r.  Evict psum, partition-shift r via
                # gpsimd, single-pass reciprocal, two multiplies.  For the
                # final tile's last pair this is the tail critical path, so
                # run it in 128-column slices that unblock proj(tt) early.
                ya_sb = rp.tile([128, QBLK], F32, tag="ya", name="ya_sb")
                yb_sb = rp.tile([128, QBLK], F32, tag="yb", name="yb_sb")
                rsh = rp.tile([128, QBLK], F32, tag="rsh", name="rsh")
                rec = rp.tile([128, QBLK], F32, tag="rec", name="rec")
                tail = (q == NQ - 1 and p == NP - 1)
                n_sl = 4 if tail else 1
                for sl in range(n_sl):
                    cs = slice(sl * (QBLK // n_sl), (sl + 1) * (QBLK // n_sl))
                    nc.vector.tensor_copy(ya_sb[:, cs], ya[:, cs])
                    if tail:
                        # Act is idle once the last exp retires; use it for
                        # the second eviction to shorten the tail chain
                        nc.scalar.copy(yb_sb[:, cs], yb[:, cs])
                    else:
                        nc.vector.tensor_copy(yb_sb[:, cs], yb[:, cs])
                    nc.gpsimd.tensor_copy(rsh[0:64, cs], ya_sb[64:128, cs])
                    nc.gpsimd.tensor_copy(rsh[64:128, cs], yb_sb[0:64, cs])
                    nc.vector.reciprocal_approx_fast(rec[:, cs], rsh[:, cs])
                    nc.vector.tensor_mul(yt[0:64, p, cs], ya_sb[0:64, cs],
                                         rec[0:64, cs])
                    nc.vector.tensor_mul(yt[64:128, p, cs],
                                         yb_sb[64:128, cs],
                                         rec[64:128, cs])
            while emitted < len(fillers):
                fillers[emitted]()
                emitted += 1
            proj_queue += [proj_group(yt, qs, tt)
                           for tt in range(QBLK // KBLK)]
            if q + 1 < NQ:
                xtq, qAB = nxtq, nqAB
        for g in proj_queue:
            g()
    nc.compile()
    return nc


def make_in_map(x_b, w_attn, b_attn, w_proj, g):
    """Per-core input arrays for batch slice x_b and head-group g."""
    sl = slice(g * GC, (g + 1) * GC)
    perm = _qk_perm()
    wq = w_attn[:, 0 * C:1 * C][:, sl][:, perm]
    wk = w_attn[:, 1 * C:2 * C][:, sl][:, perm]
    wv = w_attn[:, 2 * C:3 * C][:, sl]
    bq = b_attn[0 * C:1 * C][sl][perm]
    bk = b_attn[1 * C:2 * C][sl][perm]
    # [128, 2, 4]: per-partition bias for the 4 Q/K psum tiles (96 rows each)
    bqk = np.zeros((128, 2, 4), dtype=np.float32)
    for which, bv in enumerate((bq, bk)):
        for t in range(4):
            bqk[0:96, which, t] = bv[96 * t:96 * (t + 1)]
    import ml_dtypes
    return {
        "xt": np.ascontiguousarray(x_b.T).astype(ml_dtypes.bfloat16),
        "wa": np.ascontiguousarray(
            np.concatenate([wq, wk, wv], axis=1)).astype(ml_dtypes.bfloat16),
        "bqk": bqk,
        "wp": np.ascontiguousarray(w_proj[sl, :]),
    }


_NC_CACHE = {}


def _get_nc(T):
    if T not in _NC_CACHE:
        _NC_CACHE[T] = build_nc(T)
    return _NC_CACHE[T]


def kernel(x, w_attn, b_attn, w_proj, b_proj, _trace=False):
    from concourse.bass_utils import run_bass_kernel_spmd

    x = np.asarray(x, dtype=np.float32)
    w_attn = np.asarray(w_attn, dtype=np.float32)
    b_attn = np.asarray(b_attn, dtype=np.float32)
    w_proj = np.asarray(w_proj, dtype=np.float32)
    b_proj = np.asarray(b_proj, dtype=np.float32)
    B, T, _ = x.shape

    nc = _get_nc(T)
    in_maps = []
    for b in range(B):
        for g in range(2):
            in_maps.append(make_in_map(x[b], w_attn, b_attn, w_proj, g))
    res = run_bass_kernel_spmd(nc, in_maps, core_ids=list(range(2 * B)),
                               trace=_trace)
    outs = [np.asarray(r["out"], dtype=np.float32) for r in res.results]
    # softmax rows sum to 1, so the V-bias contribution is exactly
    # bv @ w_proj added to every token (not computed on device).
    bias_row = b_proj + b_attn[2 * C:3 * C] @ w_proj
    out = np.empty((B, T, C), dtype=np.float32)
    for b in range(B):
        out[b] = outs[2 * b] + outs[2 * b + 1] + bias_row[None, :]
    if _trace:
        kernel.last_result = res
    return out


# revision 53
# speedup vs baseline: 1.2385x; 1.0006x over previous
"""Causal self-attention (GPT-style, B=4 T=2048 C=768 H=12) on 8 trn2 cores.

Sharding: core = (batch b, head-group g), g in {0,1} covering 6 heads.
Each core: qkv projections for its 6 heads, causal flash-style attention,
partial c_proj over its 384 contraction rows; host adds the two partials
per batch plus the analytic bias row.

Key device-side structure (per core):
  x^T and w_attn ship as bf16 (halves DMA); output ships as bf16.
  QKV projections in bf16 via four 96-channel psum tiles per Q and K; a
  host-side w_attn column permutation puts 3 heads' d-halves per tile so
  Q/K evict to fp8e4 in a [32*(h%3), 2(d-half), T] pair layout with
  partition-identity copies.  All 6 heads then run S^T = K Q^T as fp8
  DoubleRow matmuls (half cycles/col; operand base partitions are
  restricted to {0,32,64}, hence 3 heads per 96-partition tile pair).
  exp on ScalarE over both heads of a pair at once ([128, 2, w] from a
  2-bank psum tile) -> P in bf16; causal diagonal masked post-exp by a
  DVE multiply.
  [V_h | 1] interleaved bf16 matmul accumulates y^T (64 partitions) and
  softmax row-sums (other 64) per (head, k-tile) into one psum bank; the
  ones blocks are memset per k-tile, V evicted with strided copies.
  Normalize: evict psum, gpsimd partition-shift of the row-sums, one
  fast reciprocal, two multiplies -> y^T fp32 (column-sliced for the
  final pair so the tail c_proj starts early).
  proj: out[t,e] = sum_f y^T[f,t] wp[f,e] in fp32r.

Scheduling: engines execute their streams in order, so PV(k-1) is issued
after S(k) -- the PE stream never blocks on exp(k) -- and cross-phase
matmul groups (next tile's QKV/V, deferred c_proj) are interleaved as
fillers into the attention loop, skewed toward the later (Act-bound)
tiles.  Startup streams (wa[ct], x[ct]) pairs on separate DGE queues so
the first QKV matmul fires as early as possible.

fp8 S gives ~1.16e-2 end-to-end rel err (vs 2.2e-3 all-bf16; gate 2e-2).
fp8 for QKV inputs, P, or V was measured at 3.7e-2..4.6e-2 -- rejected.
"""

from contextlib import ExitStack

import numpy as np

import concourse.bass as bass
import concourse.mybir as mybir
import concourse.tile as tile
from concourse import bacc
from concourse.masks import make_upper_triangular

AF = mybir.ActivationFunctionType
F32 = mybir.dt.float32
F32R = mybir.dt.float32r
BF16 = mybir.dt.bfloat16
F8 = mybir.dt.float8e4
DR = mybir.MatmulPerfMode.DoubleRow

C = 768          # model dim
D = 64           # head dim
HG = 6           # heads per core
NP = 3           # head pairs per core
GC = HG * D      # 384 group channels
CT = C // 128    # 6 contraction tiles
QBLK = 512       # query tile (psum bank)
KBLK = 128       # key tile (partition dim)

# All 6 heads run the S matmul as fp8e4 DoubleRow (half cycles/col).
# ~1.1e-2 end-to-end rel err (vs 2.2e-3 all-bf16); gate is 2e-2.
# Matmul operand base partitions are limited to {0,32,64}, so the Q/K
# channels are grouped 3 heads per 96-channel psum tile: tile t holds
# d-half (t%2) of heads 3*(t//2)..3*(t//2)+2.


def _qk_perm():
    """Channel permutation (within the 384 group channels) for Q and K.
    perm[n] = original channel feeding new channel n."""
    perm = np.empty(GC, dtype=np.int64)
    for n in range(GC):
        t, slot = divmod(n, 96)
        head = 3 * (t // 2) + slot // 32
        dd = (slot % 32) + 32 * (t % 2)
        perm[n] = head * D + dd
    return perm


_REGIONS = []      # (label, next_instruction_index) probes for trace analysis


def _mark(nc, label):
    _REGIONS.append((label,
                     int(nc.get_next_instruction_name().split("-")[-1])))


def build_nc(T=2048):
    NQ = T // QBLK
    NK = T // KBLK
    nc = bacc.Bacc(None)

    xt_d = nc.dram_tensor("xt", [C, T], BF16, kind="ExternalInput")
    wa_d = nc.dram_tensor("wa", [C, 3 * GC], BF16, kind="ExternalInput")
    bqk_d = nc.dram_tensor("bqk", [128, 2, 4], F32, kind="ExternalInput")
    wp_d = nc.dram_tensor("wp", [GC, C], F32R, kind="ExternalInput")
    out_d = nc.dram_tensor("out", [T, C], BF16, kind="ExternalOutput")

    with ExitStack() as ctx:
        tc = ctx.enter_context(tile.TileContext(nc))
        const = ctx.enter_context(tc.tile_pool(name="const", bufs=1))
        big = ctx.enter_context(tc.tile_pool(name="big", bufs=1))
        xtp = ctx.enter_context(tc.tile_pool(name="xtp", bufs=2))
        qtp = ctx.enter_context(tc.tile_pool(name="qtp", bufs=2))
        ytp = ctx.enter_context(tc.tile_pool(name="ytp", bufs=3))
        ptp = ctx.enter_context(tc.tile_pool(name="ptp", bufs=6))
        rp = ctx.enter_context(tc.tile_pool(name="rp", bufs=2))
        obp = ctx.enter_context(tc.tile_pool(name="obp", bufs=2))
        psS = ctx.enter_context(tc.tile_pool(name="psS", bufs=2, space="PSUM"))
        psY = ctx.enter_context(tc.tile_pool(name="psY", bufs=2, space="PSUM"))
        psB = ctx.enter_context(tc.tile_pool(name="psB", bufs=2, space="PSUM"))

        # causal mask, replicated for the two heads of an exp pair
        # (built after the startup DMAs are issued -- see q == 0 below --
        # so the gpsimd queue isn't busy ahead of the SWDGE x fetch)
        mask2 = const.tile([128, 2, KBLK], BF16)
        bqk_sb = const.tile([128, 2, 4], F32)

        wa = big.tile([128, CT, 3 * GC], BF16)
        wp = big.tile([128, NP, C], F32R)
        # K in fp8 d-half-pair layout: [32*(h%3)+d%32, h//3, d//32, token]
        kAB = big.tile([128, 2, 2, T], F8)
        # V interleaved with ones columns: even head h -> [V_h | 1],
        # odd head h -> [1 | V_h]; a single M=128 matmul then yields
        # y^T on one 64-partition half and the exp row-sums on the other.
        vs = big.tile([128, NK, HG, 2 * D], BF16)

        xt_r = xt_d[:, :].rearrange("(ct r) t -> ct r t", r=128)
        wa_r = wa_d[:, :].rearrange("(ct r) j -> ct r j", r=128)
        wp_r = wp_d[:, :].rearrange("(p r) e -> p r e", r=128)

        def dma_xtq(xtq, qs):
            for ct in range(CT):
                nc.sync.dma_start(out=xtq[:, ct, :],
                                  in_=xt_r[ct][:, qs:qs + QBLK])

        def qk_group(xtq, qAB, qs, which, t):
            """One Q-or-K 96-channel psum tile: 6 matmuls + fp8 eviction,
            split into two half-closures so the interleaver can place them
            at sub-group granularity."""
            cell = {}

            def half(lo, hi, evict):
                def mms():
                    _mark(nc, f"qk.w{which}.t{t}")
                    if lo == 0:
                        cell["pqk"] = psB.tile([128, QBLK], F32, tag="b",
                                               name="pqk")
                    pqk = cell["pqk"]
                    for ct in range(lo, hi):
                        nc.tensor.matmul(
                            pqk[0:96, :],
                            lhsT=wa[:, ct, which * GC + t * 96:
                                           which * GC + (t + 1) * 96],
                            rhs=xtq[:, ct, :],
                            start=(ct == 0), stop=(ct == CT - 1))
                    if evict:
                        sc = bqk_sb[0:96, which, t:t + 1]
                        g, j = divmod(t, 2)
                        if which == 0:
                            dest = qAB[0:96, g, j, :]
                        else:
                            dest = kAB[0:96, g, j, qs:qs + QBLK]
                        nc.vector.tensor_scalar_add(dest, pqk[0:96, :], sc)
                return mms
            return [half(0, 3, False), half(3, CT, True)]

        def v_group(xtq, k_i, kl):
            def mms():
                _mark(nc, f"v.k{k_i}")
                pv = psB.tile([128, QBLK], F32, tag="b", name="pv")
                for ct in range(CT):
                    nc.tensor.matmul(
                        pv[:, 0:GC],
                        lhsT=xtq[:, ct, kl * KBLK:(kl + 1) * KBLK],
                        rhs=wa[:, ct, 2 * GC:3 * GC],
                        start=(ct == 0), stop=(ct == CT - 1))
                pv3 = pv[:, 0:GC].rearrange("r (a b d) -> r a b d", b=2, d=D)
                vsv = vs[:, k_i].rearrange("r (a b) c -> r a b c", b=2)
                vso = vs[:, k_i].rearrange("r (a b) c -> r a (b c)", b=2)
                # ones occupy the middle 128 cols of each pair's 256 block;
                # even head V -> cols 0:64, odd head V -> cols 64:128 of its
                # own block
                nc.gpsimd.memset(vso[:, :, D:3 * D], 1.0)
                nc.vector.tensor_copy(vsv[:, :, 0, 0:D], pv3[:, :, 0, :])
                nc.vector.tensor_copy(vsv[:, :, 1, D:2 * D], pv3[:, :, 1, :])
            return mms

        def proj_group(yt, qs, tt):
            def mms():
                _mark(nc, f"proj.tt{tt}")
                t0 = qs + tt * KBLK
                ob = obp.tile([128, C], BF16, tag="ob", name="ob")
                for ec in range(2):
                    po = psB.tile([128, QBLK], F32, tag="b", name="po")
                    for j in range(NP):
                        nc.tensor.matmul(
                            po[:, 0:GC],
                            lhsT=yt[:, j, tt * KBLK:(tt + 1) * KBLK],
                            rhs=wp[:, j, ec * GC:(ec + 1) * GC],
                            start=(j == 0), stop=(j == NP - 1))
                    nc.vector.tensor_copy(ob[:, ec * GC:(ec + 1) * GC],
                                           po[:, 0:GC])
                q_eng = nc.sync if tt % 2 == 0 else nc.scalar
                q_eng.dma_start(out=out_d[t0:t0 + KBLK, :], in_=ob)
            return mms

        proj_queue = []         # deferred c_proj groups of earlier q-tiles
        carry = []              # fillers deferred to the next tile
        for q in range(NQ):
            qs = q * QBLK
            if q == 0:
                # startup: the QKV matmul for contraction tile ct needs the
                # (wa-qk[ct], xtq[ct]) pair, so stream those as interleaved
                # pairs on the two HWDGE queues; everything else follows.
                xtq = xtp.tile([128, CT, QBLK], BF16, tag="xtq", name="xtq")
                for ct in range(CT):
                    nc.scalar.dma_start(out=wa[:, ct, 0:2 * GC],
                                        in_=wa_r[ct][:, 0:2 * GC])
                    nc.gpsimd.dma_start(out=xtq[:, ct, :],
                                        in_=xt_r[ct][:, 0:QBLK])
                    if ct == 0:
                        nc.sync.dma_start(out=bqk_sb, in_=bqk_d[:, :, :])
                for ct in range(CT):
                    nc.scalar.dma_start(out=wa[:, ct, 2 * GC:3 * GC],
                                        in_=wa_r[ct][:, 2 * GC:3 * GC])
                pref_xtq = xtp.tile([128, CT, QBLK], BF16, tag="xtq",
                                    name="xtq")
                for ct in range(CT):
                    nc.gpsimd.dma_start(out=pref_xtq[:, ct, :],
                                        in_=xt_r[ct][:, QBLK:2 * QBLK])
                for pp in range(NP):
                    nc.scalar.dma_start(out=wp[:, pp, :], in_=wp_r[pp])
                make_upper_triangular(nc, mask2[:, 0, :], val=1.0, diag=True)
                make_upper_triangular(nc, mask2[:, 1, :], val=1.0, diag=True)
                qAB = qtp.tile([128, 2, 2, QBLK], F8, tag="qAB", name="qAB")
                for which in (0, 1):
                    for t in range(4):
                        for h in qk_group(xtq, qAB, qs, which, t):
                            h()
                for k_i in range(4):
                    v_group(xtq, k_i, k_i)()

            # fillers interleaved into this q-tile's attention stream.  The
            # early tiles are PE-rich (small attention) and the late ones
            # Act-bound, so PE-side work is skewed late: the last tile gets
            # the K-projection of its own QKV plus two tiles' worth of
            # c_proj from the deferred queue.
            fillers = list(carry)
            carry = []
            if q + 1 < NQ:
                nqs = qs + QBLK
                if q == 0:
                    nxtq = pref_xtq
                else:
                    nxtq = xtp.tile([128, CT, QBLK], BF16, tag="xtq",
                                    name="xtq")
                    dma_xtq(nxtq, nqs)
                nqAB = qtp.tile([128, 2, 2, QBLK], F8, tag="qAB", name="qAB")
                for t in range(4):
                    fillers += qk_group(nxtq, nqAB, nqs, 0, t)
                kq = [h for t in range(4)
                      for h in qk_group(nxtq, nqAB, nqs, 1, t)]
                if q + 1 == NQ - 1:
                    carry += kq      # K of the last tile: emit during it
                else:
                    fillers += kq
                for kl in range(4):
                    fillers.append(v_group(nxtq, 4 * (q + 1) + kl, kl))
            # deferred c_proj: none before att(2); proj(0) at att(2);
            # the rest at att(3)
            if q == NQ - 1:
                fillers += proj_queue
                proj_queue = []

            nkt = (q + 1) * (QBLK // KBLK)
            n_slots = NP * nkt
            yt = ytp.tile([128, NP, QBLK], F32R, tag="yt", name="yt")
            slot = 0
            emitted = 0
            for p in range(NP):
                ya = psY.tile([128, QBLK], F32, tag="y", name="ya")
                yb = psY.tile([128, QBLK], F32, tag="y", name="yb")
                pend = None     # software-pipelined PV of the previous k_i
                for k_i in range(nkt):
                    _mark(nc, f"att.q{q}.p{p}.k{k_i}")
                    m = k_i - 4 * q
                    col0 = max(m, 0) * KBLK
                    st2 = psS.tile([128, 2, QBLK], F32, tag="st", name="st2")
                    pt2 = ptp.tile([128, 2, QBLK], BF16, tag="pt", name="pt2")
                    for s in range(2):
                        h = 2 * p + s
                        g, hb = divmod(h, 3)
                        base = 32 * hb
                        nc.tensor.matmul(
                            st2[:, s, col0:QBLK],
                            lhsT=kAB[base:base + 32, g, :,
                                     k_i * KBLK:(k_i + 1) * KBLK],
                            rhs=qAB[base:base + 32, g, :, col0:QBLK],
                            start=True, stop=True, perf_mode=DR)
                    nc.scalar.activation(pt2[:, :, col0:QBLK],
                                         st2[:, :, col0:QBLK],
                                         AF.Exp, scale=0.125)
                    if m >= 0:
                        seg = pt2[:, :, col0:col0 + KBLK]
                        nc.vector.tensor_mul(seg, seg, mask2)
                    if pend is not None:
                        pend()
                    first = (k_i == 0)
                    last = (k_i == nkt - 1)

                    def make_pv(pt2=pt2, p=p, col0=col0, first=first,
                                last=last, k_i=k_i, ya=ya, yb=yb):
                        def pv():
                            for s in range(2):
                                yy = ya if s == 0 else yb
                                nc.tensor.matmul(
                                    yy[:, col0:QBLK],
                                    lhsT=vs[:, k_i, 2 * p + s, :],
                                    rhs=pt2[:, s, col0:QBLK],
                                    start=first, stop=last,
                                    skip_group_check=True)
                        return pv
                    pend = make_pv()
                    # interleave cross-phase matmul groups (front-loaded so
                    # dependency-critical groups land before their readers)
                    slot += 1
                    want = min(len(fillers),
                               (3 * slot * len(fillers)) // (2 * n_slots))
                    while emitted < want:
                        fillers[emitted]()
                        emitted += 1
                pend()
                _mark(nc, f"norm.q{q}.p{p}")
                # normalize: y^T / r.  Evict psum, partition-shift r via
                # gpsimd, single-pass reciprocal, two multiplies.  For the
                # final tile's last pair this is the tail critical path, so
                # run it in 128-column slices that unblock proj(tt) early.
                ya_sb = rp.tile([128, QBLK], F32, tag="ya", name="ya_sb")
                yb_sb = rp.tile([128, QBLK], F32, tag="yb", name="yb_sb")
                rsh = rp.tile([128, QBLK], F32, tag="rsh", name="rsh")
                rec = rp.tile([128, QBLK], F32, tag="rec", name="rec")
                tail = (q == NQ - 1 and p == NP - 1)
                n_sl = 4 if tail else 1
                for sl in range(n_sl):
                    cs = slice(sl * (QBLK // n_sl), (sl + 1) * (QBLK // n_sl))
                    nc.vector.tensor_copy(ya_sb[:, cs], ya[:, cs])
                    if tail:
                        # Act is idle once the last exp retires; use it for
                        # the second eviction to shorten the tail chain
                        nc.scalar.copy(yb_sb[:, cs], yb[:, cs])
                    else:
                        nc.vector.tensor_copy(yb_sb[:, cs], yb[:, cs])
                    nc.gpsimd.tensor_copy(rsh[0:64, cs], ya_sb[64:128, cs])
                    nc.gpsimd.tensor_copy(rsh[64:128, cs], yb_sb[0:64, cs])
                    nc.vector.reciprocal_approx_fast(rec[:, cs], rsh[:, cs])
                    nc.vector.tensor_mul(yt[0:64, p, cs], ya_sb[0:64, cs],
                                         rec[0:64, cs])
                    nc.vector.tensor_mul(yt[64:128, p, cs],
                                         yb_sb[64:128, cs],
                                         rec[64:128, cs])
            while emitted < len(fillers):
                fillers[emitted]()
                emitted += 1
            proj_queue += [proj_group(yt, qs, tt)
                           for tt in range(QBLK // KBLK)]
            if q + 1 < NQ:
                xtq, qAB = nxtq, nqAB
        for g in proj_queue:
            g()
    nc.compile()
    return nc


def make_in_map(x_b, w_attn, b_attn, w_proj, g):
    """Per-core input arrays for batch slice x_b and head-group g."""
    sl = slice(g * GC, (g + 1) * GC)
    perm = _qk_perm()
    wq = w_attn[:, 0 * C:1 * C][:, sl][:, perm]
    wk = w_attn[:, 1 * C:2 * C][:, sl][:, perm]
    wv = w_attn[:, 2 * C:3 * C][:, sl]
    bq = b_attn[0 * C:1 * C][sl][perm]
    bk = b_attn[1 * C:2 * C][sl][perm]
    # [128, 2, 4]: per-partition bias for the 4 Q/K psum tiles (96 rows each)
    bqk = np.zeros((128, 2, 4), dtype=np.float32)
    for which, bv in enumerate((bq, bk)):
        for t in range(4):
            bqk[0:96, which, t] = bv[96 * t:96 * (t + 1)]
    import ml_dtypes
    return {
        "xt": np.ascontiguousarray(x_b.T).astype(ml_dtypes.bfloat16),
        "wa": np.ascontiguousarray(
            np.concatenate([wq, wk, wv], axis=1)).astype(ml_dtypes.bfloat16),
        "bqk": bqk,
        "wp": np.ascontiguousarray(w_proj[sl, :]),
    }


_NC_CACHE = {}


def _get_nc(T):
    if T not in _NC_CACHE:
        _NC_CACHE[T] = build_nc(T)
    return _NC_CACHE[T]


def kernel(x, w_attn, b_attn, w_proj, b_proj, _trace=False):
    from concourse.bass_utils import run_bass_kernel_spmd

    x = np.asarray(x, dtype=np.float32)
    w_attn = np.asarray(w_attn, dtype=np.float32)
    b_attn = np.asarray(b_attn, dtype=np.float32)
    w_proj = np.asarray(w_proj, dtype=np.float32)
    b_proj = np.asarray(b_proj, dtype=np.float32)
    B, T, _ = x.shape

    nc = _get_nc(T)
    in_maps = []
    for b in range(B):
        for g in range(2):
            in_maps.append(make_in_map(x[b], w_attn, b_attn, w_proj, g))
    res = run_bass_kernel_spmd(nc, in_maps, core_ids=list(range(2 * B)),
                               trace=_trace)
    outs = [np.asarray(r["out"], dtype=np.float32) for r in res.results]
    # softmax rows sum to 1, so the V-bias contribution is exactly
    # bv @ w_proj added to every token (not computed on device).
    bias_row = b_proj + b_attn[2 * C:3 * C] @ w_proj
    out = np.empty((B, T, C), dtype=np.float32)
    for b in range(B):
        out[b] = outs[2 * b] + outs[2 * b + 1] + bias_row[None, :]
    if _trace:
        kernel.last_result = res
    return out


# revision 55
# speedup vs baseline: 1.2834x; 1.0362x over previous
"""Causal self-attention (GPT-style, B=4 T=2048 C=768 H=12) on 8 trn2 cores.

Sharding: core = (batch b, head-group g), g in {0,1} covering 6 heads.
Each core: qkv projections for its 6 heads, causal flash-style attention,
partial c_proj over its 384 contraction rows; host adds the two partials
per batch plus the analytic bias row.

Key device-side structure (per core):
  x^T and w_attn ship as bf16 (halves DMA); output ships as bf16.
  QKV projections in bf16 via four 96-channel psum tiles per Q and K; a
  host-side w_attn column permutation puts 3 heads' d-halves per tile so
  Q/K evict to fp8e4 in a [32*(h%3), 2(d-half), T] pair layout with
  partition-identity copies.  All 6 heads then run S^T = K Q^T as fp8
  DoubleRow matmuls (half cycles/col; operand base partitions are
  restricted to {0,32,64}, hence 3 heads per 96-partition tile pair).
  exp on ScalarE over both heads of a pair at once ([128, 2, w] from a
  2-bank psum tile) -> P in bf16; causal diagonal masked post-exp by a
  DVE multiply.
  [V_h | 1] interleaved bf16 matmul accumulates y^T (64 partitions) and
  softmax row-sums (other 64) per (head, k-tile) into one psum bank; the
  ones blocks are memset per k-tile, V evicted with strided copies.
  Normalize: evict psum, gpsimd partition-shift of the row-sums, one
  fast reciprocal, two multiplies -> y^T fp32 (column-sliced for the
  final pair so the tail c_proj starts early).
  proj: out[t,e] = sum_f y^T[f,t] wp[f,e] in fp32r.

Scheduling: engines execute their streams in order, so PV(k-1) is issued
after S(k) -- the PE stream never blocks on exp(k) -- and cross-phase
matmul groups (next tile's QKV/V, deferred c_proj) are interleaved as
fillers into the attention loop, skewed toward the later (Act-bound)
tiles.  Startup streams (wa[ct], x[ct]) pairs on separate DGE queues so
the first QKV matmul fires as early as possible.

fp8 S gives ~1.16e-2 end-to-end rel err (vs 2.2e-3 all-bf16; gate 2e-2).
fp8 for QKV inputs, P, or V was measured at 3.7e-2..4.6e-2 -- rejected.
"""

from contextlib import ExitStack

import numpy as np

import concourse.bass as bass
import concourse.mybir as mybir
import concourse.tile as tile
from concourse import bacc
from concourse.masks import make_upper_triangular

AF = mybir.ActivationFunctionType
F32 = mybir.dt.float32
F32R = mybir.dt.float32r
BF16 = mybir.dt.bfloat16
F8 = mybir.dt.float8e4
DR = mybir.MatmulPerfMode.DoubleRow

C = 768          # model dim
D = 64           # head dim
HG = 6           # heads per core
NP = 3           # head pairs per core
GC = HG * D      # 384 group channels
CT = C // 128    # 6 contraction tiles
QBLK = 512       # query tile (psum bank)
KBLK = 128       # key tile (partition dim)

# All 6 heads run the S matmul as fp8e4 DoubleRow (half cycles/col).
# ~1.1e-2 end-to-end rel err (vs 2.2e-3 all-bf16); gate is 2e-2.
# Matmul operand base partitions are limited to {0,32,64}, so the Q/K
# channels are grouped 3 heads per 96-channel psum tile: tile t holds
# d-half (t%2) of heads 3*(t//2)..3*(t//2)+2.


def _qk_perm():
    """Channel permutation (within the 384 group channels) for Q and K.
    Tile 0 holds d0-31 of heads 0-3, tile 1 d32-63 of heads 0-3, tile 2
    heads 4-5 in natural d-major order.  perm[n] = original channel
    feeding new channel n."""
    perm = np.empty(GC, dtype=np.int64)
    for n in range(GC):
        t, slot = divmod(n, 128)
        if t < 2:
            head, dd = divmod(slot, 32)
            perm[n] = head * D + 32 * t + dd
        else:
            head, dd = divmod(slot, 64)
            perm[n] = (4 + head) * D + dd
    return perm


_REGIONS = []      # (label, next_instruction_index) probes for trace analysis


def _mark(nc, label):
    _REGIONS.append((label,
                     int(nc.get_next_instruction_name().split("-")[-1])))


def build_nc(T=2048):
    NQ = T // QBLK
    NK = T // KBLK
    nc = bacc.Bacc(None)

    xt_d = nc.dram_tensor("xt", [C, T], BF16, kind="ExternalInput")
    wa_d = nc.dram_tensor("wa", [C, 3 * GC], BF16, kind="ExternalInput")
    bqk_d = nc.dram_tensor("bqk", [128, 2, 3], F32, kind="ExternalInput")
    wp_d = nc.dram_tensor("wp", [GC, C], F32R, kind="ExternalInput")
    out_d = nc.dram_tensor("out", [T, C], BF16, kind="ExternalOutput")

    with ExitStack() as ctx:
        tc = ctx.enter_context(tile.TileContext(nc))
        const = ctx.enter_context(tc.tile_pool(name="const", bufs=1))
        big = ctx.enter_context(tc.tile_pool(name="big", bufs=1))
        xtp = ctx.enter_context(tc.tile_pool(name="xtp", bufs=2))
        qtp = ctx.enter_context(tc.tile_pool(name="qtp", bufs=2))
        ytp = ctx.enter_context(tc.tile_pool(name="ytp", bufs=3))
        ptp = ctx.enter_context(tc.tile_pool(name="ptp", bufs=6))
        rp = ctx.enter_context(tc.tile_pool(name="rp", bufs=2))
        obp = ctx.enter_context(tc.tile_pool(name="obp", bufs=2))
        psS = ctx.enter_context(tc.tile_pool(name="psS", bufs=2, space="PSUM"))
        psY = ctx.enter_context(tc.tile_pool(name="psY", bufs=2, space="PSUM"))
        psB = ctx.enter_context(tc.tile_pool(name="psB", bufs=2, space="PSUM"))

        # causal mask, replicated for the two heads of an exp pair
        # (built after the startup DMAs are issued -- see q == 0 below --
        # so the gpsimd queue isn't busy ahead of the SWDGE x fetch)
        mask2 = const.tile([128, 2, KBLK], BF16)
        bqk_sb = const.tile([128, 2, 3], F32)

        wa = big.tile([128, CT, 3 * GC], BF16)
        wp = big.tile([128, NP, C], F32R)
        # K for heads 0-3 in fp8 d-half-pair layout: [32h+d%32, d//32, tok]
        kAB = big.tile([128, 2, T], F8)
        # K for heads 4,5: bf16 d-major
        kC = big.tile([128, T], BF16)
        # V interleaved with ones columns: even head h -> [V_h | 1],
        # odd head h -> [1 | V_h]; a single M=128 matmul then yields
        # y^T on one 64-partition half and the exp row-sums on the other.
        vs = big.tile([128, NK, HG, 2 * D], BF16)

        xt_r = xt_d[:, :].rearrange("(ct r) t -> ct r t", r=128)
        wa_r = wa_d[:, :].rearrange("(ct r) j -> ct r j", r=128)
        wp_r = wp_d[:, :].rearrange("(p r) e -> p r e", r=128)

        def dma_xtq(xtq, qs):
            for ct in range(CT):
                nc.sync.dma_start(out=xtq[:, ct, :],
                                  in_=xt_r[ct][:, qs:qs + QBLK])

        def qk_group(xtq, qAB, qC, qs, which, t):
            """One Q-or-K 128-channel psum tile: 6 matmuls + eviction
            (fp8 pair layout for tiles 0-1, bf16 d-major for tile 2),
            split into two half-closures so the interleaver can place
            them at sub-group granularity."""
            cell = {}

            def half(lo, hi, evict):
                def mms():
                    _mark(nc, f"qk.w{which}.t{t}")
                    if lo == 0:
                        cell["pqk"] = psB.tile([128, QBLK], F32, tag="b",
                                               name="pqk")
                    pqk = cell["pqk"]
                    for ct in range(lo, hi):
                        nc.tensor.matmul(
                            pqk,
                            lhsT=wa[:, ct, which * GC + t * 128:
                                           which * GC + (t + 1) * 128],
                            rhs=xtq[:, ct, :],
                            start=(ct == 0), stop=(ct == CT - 1))
                    if evict:
                        sc = bqk_sb[:, which, t:t + 1]
                        if t < 2:
                            dest = qAB[:, t, :] if which == 0 \
                                else kAB[:, t, qs:qs + QBLK]
                        else:
                            dest = qC if which == 0 \
                                else kC[:, qs:qs + QBLK]
                        nc.vector.tensor_scalar_add(dest, pqk, sc)
                return mms
            return [half(0, 3, False), half(3, CT, True)]

        def v_group(xtq, k_i, kl):
            def mms():
                _mark(nc, f"v.k{k_i}")
                pv = psB.tile([128, QBLK], F32, tag="b", name="pv")
                for ct in range(CT):
                    nc.tensor.matmul(
                        pv[:, 0:GC],
                        lhsT=xtq[:, ct, kl * KBLK:(kl + 1) * KBLK],
                        rhs=wa[:, ct, 2 * GC:3 * GC],
                        start=(ct == 0), stop=(ct == CT - 1))
                pv3 = pv[:, 0:GC].rearrange("r (a b d) -> r a b d", b=2, d=D)
                vsv = vs[:, k_i].rearrange("r (a b) c -> r a b c", b=2)
                vso = vs[:, k_i].rearrange("r (a b) c -> r a (b c)", b=2)
                # ones occupy the middle 128 cols of each pair's 256 block;
                # even head V -> cols 0:64, odd head V -> cols 64:128 of its
                # own block
                nc.gpsimd.memset(vso[:, :, D:3 * D], 1.0)
                nc.vector.tensor_copy(vsv[:, :, 0, 0:D], pv3[:, :, 0, :])
                nc.vector.tensor_copy(vsv[:, :, 1, D:2 * D], pv3[:, :, 1, :])
            return mms

        def proj_group(yt, qs, tt):
            def mms():
                _mark(nc, f"proj.tt{tt}")
                t0 = qs + tt * KBLK
                ob = obp.tile([128, C], BF16, tag="ob", name="ob")
                for ec in range(2):
                    po = psB.tile([128, QBLK], F32, tag="b", name="po")
                    for j in range(NP):
                        nc.tensor.matmul(
                            po[:, 0:GC],
                            lhsT=yt[:, j, tt * KBLK:(tt + 1) * KBLK],
                            rhs=wp[:, j, ec * GC:(ec + 1) * GC],
                            start=(j == 0), stop=(j == NP - 1))
                    nc.vector.tensor_copy(ob[:, ec * GC:(ec + 1) * GC],
                                           po[:, 0:GC])
                q_eng = nc.sync if tt % 2 == 0 else nc.scalar
                q_eng.dma_start(out=out_d[t0:t0 + KBLK, :], in_=ob)
            return mms

        proj_queue = []         # deferred c_proj groups of earlier q-tiles
        carry = []              # fillers deferred to the next tile
        for q in range(NQ):
            qs = q * QBLK
            if q == 0:
                # startup: the QKV matmul for contraction tile ct needs the
                # (wa-qk[ct], xtq[ct]) pair, so stream those as interleaved
                # pairs on the two HWDGE queues; everything else follows.
                xtq = xtp.tile([128, CT, QBLK], BF16, tag="xtq", name="xtq")
                for ct in range(CT):
                    nc.scalar.dma_start(out=wa[:, ct, 0:2 * GC],
                                        in_=wa_r[ct][:, 0:2 * GC])
                    nc.gpsimd.dma_start(out=xtq[:, ct, :],
                                        in_=xt_r[ct][:, 0:QBLK])
                    if ct == 0:
                        nc.sync.dma_start(out=bqk_sb, in_=bqk_d[:, :, :])
                for ct in range(CT):
                    nc.scalar.dma_start(out=wa[:, ct, 2 * GC:3 * GC],
                                        in_=wa_r[ct][:, 2 * GC:3 * GC])
                pref_xtq = xtp.tile([128, CT, QBLK], BF16, tag="xtq",
                                    name="xtq")
                for ct in range(CT):
                    nc.gpsimd.dma_start(out=pref_xtq[:, ct, :],
                                        in_=xt_r[ct][:, QBLK:2 * QBLK])
                for pp in range(NP):
                    nc.scalar.dma_start(out=wp[:, pp, :], in_=wp_r[pp])
                make_upper_triangular(nc, mask2[:, 0, :], val=1.0, diag=True)
                make_upper_triangular(nc, mask2[:, 1, :], val=1.0, diag=True)
                qAB = qtp.tile([128, 2, QBLK], F8, tag="qAB", name="qAB")
                qC = qtp.tile([128, QBLK], BF16, tag="qC", name="qC")
                for which in (0, 1):
                    for t in range(3):
                        for h in qk_group(xtq, qAB, qC, qs, which, t):
                            h()
                for k_i in range(4):
                    v_group(xtq, k_i, k_i)()

            # fillers interleaved into this q-tile's attention stream.  The
            # early tiles are PE-rich (small attention) and the late ones
            # Act-bound, so PE-side work is skewed late: the last tile gets
            # the K-projection of its own QKV plus two tiles' worth of
            # c_proj from the deferred queue.
            fillers = list(carry)
            carry = []
            if q + 1 < NQ:
                nqs = qs + QBLK
                if q == 0:
                    nxtq = pref_xtq
                else:
                    nxtq = xtp.tile([128, CT, QBLK], BF16, tag="xtq",
                                    name="xtq")
                    dma_xtq(nxtq, nqs)
                nqAB = qtp.tile([128, 2, QBLK], F8, tag="qAB", name="qAB")
                nqC = qtp.tile([128, QBLK], BF16, tag="qC", name="qC")
                for t in range(3):
                    fillers += qk_group(nxtq, nqAB, nqC, nqs, 0, t)
                kq = [h for t in range(3)
                      for h in qk_group(nxtq, nqAB, nqC, nqs, 1, t)]
                if q + 1 == NQ - 1:
                    carry += kq      # K of the last tile: emit during it
                else:
                    fillers += kq
                for kl in range(4):
                    fillers.append(v_group(nxtq, 4 * (q + 1) + kl, kl))
            # deferred c_proj: none before att(2); proj(0) at att(2);
            # the rest at att(3)
            if q == NQ - 1:
                fillers += proj_queue
                proj_queue = []

            nkt = (q + 1) * (QBLK // KBLK)
            n_slots = NP * nkt
            yt = ytp.tile([128, NP, QBLK], F32R, tag="yt", name="yt")
            slot = 0
            emitted = 0
            for p in range(NP):
                ya = psY.tile([128, QBLK], F32, tag="y", name="ya")
                yb = psY.tile([128, QBLK], F32, tag="y", name="yb")
                pend = None     # software-pipelined PV of the previous k_i
                for k_i in range(nkt):
                    _mark(nc, f"att.q{q}.p{p}.k{k_i}")
                    m = k_i - 4 * q
                    col0 = max(m, 0) * KBLK
                    st2 = psS.tile([128, 2, QBLK], F32, tag="st", name="st2")
                    pt2 = ptp.tile([128, 2, QBLK], BF16, tag="pt", name="pt2")
                    for s in range(2):
                        h = 2 * p + s
                        if h < 4:
                            base = 32 * h
                            nc.tensor.matmul(
                                st2[:, s, col0:QBLK],
                                lhsT=kAB[base:base + 32, :,
                                         k_i * KBLK:(k_i + 1) * KBLK],
                                rhs=qAB[base:base + 32, :, col0:QBLK],
                                start=True, stop=True, perf_mode=DR,
                                tile_position=(base, 0))
                        else:
                            hb = 64 * (h - 4)
                            nc.tensor.matmul(
                                st2[:, s, col0:QBLK],
                                lhsT=kC[hb:hb + 64,
                                        k_i * KBLK:(k_i + 1) * KBLK],
                                rhs=qC[hb:hb + 64, col0:QBLK],
                                start=True, stop=True)
                    nc.scalar.activation(pt2[:, :, col0:QBLK],
                                         st2[:, :, col0:QBLK],
                                         AF.Exp, scale=0.125)
                    if m >= 0:
                        seg = pt2[:, :, col0:col0 + KBLK]
                        nc.vector.tensor_mul(seg, seg, mask2)
                    if pend is not None:
                        pend()
                    first = (k_i == 0)
                    last = (k_i == nkt - 1)

                    def make_pv(pt2=pt2, p=p, col0=col0, first=first,
                                last=last, k_i=k_i, ya=ya, yb=yb):
                        def pv():
                            for s in range(2):
                                yy = ya if s == 0 else yb
                                nc.tensor.matmul(
                                    yy[:, col0:QBLK],
                                    lhsT=vs[:, k_i, 2 * p + s, :],
                                    rhs=pt2[:, s, col0:QBLK],
                                    start=first, stop=last,
                                    skip_group_check=True)
                        return pv
                    pend = make_pv()
                    # interleave cross-phase matmul groups (front-loaded so
                    # dependency-critical groups land before their readers)
                    slot += 1
                    want = min(len(fillers),
                               (3 * slot * len(fillers)) // (2 * n_slots))
                    while emitted < want:
                        fillers[emitted]()
                        emitted += 1
                pend()
                _mark(nc, f"norm.q{q}.p{p}")
                # normalize: y^T / r.  Evict psum, partition-shift r via
                # gpsimd, single-pass reciprocal, two multiplies.  For the
                # final tile's last pair this is the tail critical path, so
                # run it in 128-column slices that unblock proj(tt) early.
                ya_sb = rp.tile([128, QBLK], F32, tag="ya", name="ya_sb")
                yb_sb = rp.tile([128, QBLK], F32, tag="yb", name="yb_sb")
                rsh = rp.tile([128, QBLK], F32, tag="rsh", name="rsh")
                rec = rp.tile([128, QBLK], F32, tag="rec", name="rec")
                tail = (q == NQ - 1 and p == NP - 1)
                n_sl = 4 if tail else 1
                for sl in range(n_sl):
                    cs = slice(sl * (QBLK // n_sl), (sl + 1) * (QBLK // n_sl))
                    nc.vector.tensor_copy(ya_sb[:, cs], ya[:, cs])
                    if tail:
                        # Act is idle once the last exp retires; use it for
                        # the second eviction to shorten the tail chain
                        nc.scalar.copy(yb_sb[:, cs], yb[:, cs])
                    else:
                        nc.vector.tensor_copy(yb_sb[:, cs], yb[:, cs])
                    nc.gpsimd.tensor_copy(rsh[0:64, cs], ya_sb[64:128, cs])
                    nc.gpsimd.tensor_copy(rsh[64:128, cs], yb_sb[0:64, cs])
                    nc.vector.reciprocal_approx_fast(rec[:, cs], rsh[:, cs])
                    nc.vector.tensor_mul(yt[0:64, p, cs], ya_sb[0:64, cs],
                                         rec[0:64, cs])
                    nc.vector.tensor_mul(yt[64:128, p, cs],
                                         yb_sb[64:128, cs],
                                         rec[64:128, cs])
            while emitted < len(fillers):
                fillers[emitted]()
                emitted += 1
            proj_queue += [proj_group(yt, qs, tt)
                           for tt in range(QBLK // KBLK)]
            if q + 1 < NQ:
                xtq, qAB, qC = nxtq, nqAB, nqC
        for g in proj_queue:
            g()
    nc.compile()
    return nc


def make_in_map(x_b, w_attn, b_attn, w_proj, g):
    """Per-core input arrays for batch slice x_b and head-group g."""
    sl = slice(g * GC, (g + 1) * GC)
    perm = _qk_perm()
    wq = w_attn[:, 0 * C:1 * C][:, sl][:, perm]
    wk = w_attn[:, 1 * C:2 * C][:, sl][:, perm]
    wv = w_attn[:, 2 * C:3 * C][:, sl]
    bq = b_attn[0 * C:1 * C][sl][perm]
    bk = b_attn[1 * C:2 * C][sl][perm]
    # [128, 2, 3]: per-partition bias for the 3 Q/K psum tiles
    bqk = np.ascontiguousarray(
        np.stack([bq, bk]).reshape(2, 3, 128).transpose(2, 0, 1))
    import ml_dtypes
    return {
        "xt": np.ascontiguousarray(x_b.T).astype(ml_dtypes.bfloat16),
        "wa": np.ascontiguousarray(
            np.concatenate([wq, wk, wv], axis=1)).astype(ml_dtypes.bfloat16),
        "bqk": bqk,
        "wp": np.ascontiguousarray(w_proj[sl, :]),
    }


_NC_CACHE = {}


def _get_nc(T):
    if T not in _NC_CACHE:
        _NC_CACHE[T] = build_nc(T)
    return _NC_CACHE[T]


def kernel(x, w_attn, b_attn, w_proj, b_proj, _trace=False):
    from concourse.bass_utils import run_bass_kernel_spmd

    x = np.asarray(x, dtype=np.float32)
    w_attn = np.asarray(w_attn, dtype=np.float32)
    b_attn = np.asarray(b_attn, dtype=np.float32)
    w_proj = np.asarray(w_proj, dtype=np.float32)
    b_proj = np.asarray(b_proj, dtype=np.float32)
    B, T, _ = x.shape

    nc = _get_nc(T)
    in_maps = []
    for b in range(B):
        for g in range(2):
            in_maps.append(make_in_map(x[b], w_attn, b_attn, w_proj, g))
    res = run_bass_kernel_spmd(nc, in_maps, core_ids=list(range(2 * B)),
                               trace=_trace)
    outs = [np.asarray(r["out"], dtype=np.float32) for r in res.results]
    # softmax rows sum to 1, so the V-bias contribution is exactly
    # bv @ w_proj added to every token (not computed on device).
    bias_row = b_proj + b_attn[2 * C:3 * C] @ w_proj
    out = np.empty((B, T, C), dtype=np.float32)
    for b in range(B):
        out[b] = outs[2 * b] + outs[2 * b + 1] + bias_row[None, :]
    if _trace:
        kernel.last_result = res
    return out


# revision 59
# speedup vs baseline: 1.3084x; 1.0195x over previous
"""Causal self-attention (GPT-style, B=4 T=2048 C=768 H=12) on 8 trn2 cores.

Sharding: core = (batch b, head-group g), g in {0,1} covering 6 heads.
Each core: qkv projections for its 6 heads, causal flash-style attention,
partial c_proj over its 384 contraction rows; host adds the two partials
per batch plus the analytic bias row.

Key device-side structure (per core):
  x^T and w_attn ship as bf16 (halves DMA); output ships as bf16.
  QKV projections in bf16 via four 96-channel psum tiles per Q and K; a
  host-side w_attn column permutation puts 3 heads' d-halves per tile so
  Q/K evict to fp8e4 in a [32*(h%3), 2(d-half), T] pair layout with
  partition-identity copies.  All 6 heads then run S^T = K Q^T as fp8
  DoubleRow matmuls (half cycles/col; operand base partitions are
  restricted to {0,32,64}, hence 3 heads per 96-partition tile pair).
  exp on ScalarE over both heads of a pair at once ([128, 2, w] from a
  2-bank psum tile) -> P in bf16; causal diagonal masked post-exp by a
  DVE multiply.
  [V_h | 1] interleaved bf16 matmul accumulates y^T (64 partitions) and
  softmax row-sums (other 64) per (head, k-tile) into one psum bank; the
  ones blocks are memset per k-tile, V evicted with strided copies.
  Normalize: evict psum, gpsimd partition-shift of the row-sums, one
  fast reciprocal, two multiplies -> y^T fp32 (column-sliced for the
  final pair so the tail c_proj starts early).
  proj: out[t,e] = sum_f y^T[f,t] wp[f,e] in fp32r.

Scheduling: engines execute their streams in order, so PV(k-1) is issued
after S(k) -- the PE stream never blocks on exp(k) -- and cross-phase
matmul groups (next tile's QKV/V, deferred c_proj) are interleaved as
fillers into the attention loop, skewed toward the later (Act-bound)
tiles.  Startup streams (wa[ct], x[ct]) pairs on separate DGE queues so
the first QKV matmul fires as early as possible.

fp8 S gives ~1.16e-2 end-to-end rel err (vs 2.2e-3 all-bf16; gate 2e-2).
fp8 for QKV inputs, P, or V was measured at 3.7e-2..4.6e-2 -- rejected.
"""

from contextlib import ExitStack

import numpy as np

import concourse.bass as bass
import concourse.mybir as mybir
import concourse.tile as tile
from concourse import bacc
from concourse.masks import make_upper_triangular

AF = mybir.ActivationFunctionType
F32 = mybir.dt.float32
F32R = mybir.dt.float32r
BF16 = mybir.dt.bfloat16
F8 = mybir.dt.float8e4
DR = mybir.MatmulPerfMode.DoubleRow

C = 768          # model dim
D = 64           # head dim
HG = 6           # heads per core
NP = 3           # head pairs per core
GC = HG * D      # 384 group channels
CT = C // 128    # 6 contraction tiles
QBLK = 512       # query tile (psum bank)
KBLK = 128       # key tile (partition dim)

# All 6 heads run the S matmul as fp8e4 DoubleRow (half cycles/col).
# ~1.1e-2 end-to-end rel err (vs 2.2e-3 all-bf16); gate is 2e-2.
# Matmul operand base partitions are limited to {0,32,64}, so the Q/K
# channels are grouped 3 heads per 96-channel psum tile: tile t holds
# d-half (t%2) of heads 3*(t//2)..3*(t//2)+2.


def _qk_perm():
    """Channel permutation (within the 384 group channels) for Q and K.
    Tile 0 holds d0-31 of heads 0-3, tile 1 d32-63 of heads 0-3, tile 2
    heads 4-5 in natural d-major order.  perm[n] = original channel
    feeding new channel n."""
    perm = np.empty(GC, dtype=np.int64)
    for n in range(GC):
        t, slot = divmod(n, 128)
        if t < 2:
            head, dd = divmod(slot, 32)
            perm[n] = head * D + 32 * t + dd
        else:
            head, dd = divmod(slot, 64)
            perm[n] = (4 + head) * D + dd
    return perm


_REGIONS = []      # (label, next_instruction_index) probes for trace analysis


def _mark(nc, label):
    _REGIONS.append((label,
                     int(nc.get_next_instruction_name().split("-")[-1])))


def build_nc(T=2048):
    NQ = T // QBLK
    NK = T // KBLK
    nc = bacc.Bacc(None)

    xt_d = nc.dram_tensor("xt", [C, T], BF16, kind="ExternalInput")
    wa_d = nc.dram_tensor("wa", [C, 3 * GC], BF16, kind="ExternalInput")
    bqk_d = nc.dram_tensor("bqk", [128, 2, 3], F32, kind="ExternalInput")
    wp_d = nc.dram_tensor("wp", [GC, C], F32R, kind="ExternalInput")
    out_d = nc.dram_tensor("out", [T, C], BF16, kind="ExternalOutput")

    with ExitStack() as ctx:
        tc = ctx.enter_context(tile.TileContext(nc))
        const = ctx.enter_context(tc.tile_pool(name="const", bufs=1))
        big = ctx.enter_context(tc.tile_pool(name="big", bufs=1))
        xtp = ctx.enter_context(tc.tile_pool(name="xtp", bufs=2))
        qtp = ctx.enter_context(tc.tile_pool(name="qtp", bufs=2))
        ytp = ctx.enter_context(tc.tile_pool(name="ytp", bufs=3))
        ptp = ctx.enter_context(tc.tile_pool(name="ptp", bufs=6))
        rp = ctx.enter_context(tc.tile_pool(name="rp", bufs=2))
        obp = ctx.enter_context(tc.tile_pool(name="obp", bufs=2))
        psS = ctx.enter_context(tc.tile_pool(name="psS", bufs=2, space="PSUM"))
        psY = ctx.enter_context(tc.tile_pool(name="psY", bufs=2, space="PSUM"))
        psB = ctx.enter_context(tc.tile_pool(name="psB", bufs=2, space="PSUM"))

        # causal mask, replicated for the two heads of an exp pair
        # (built after the startup DMAs are issued -- see q == 0 below --
        # so the gpsimd queue isn't busy ahead of the SWDGE x fetch)
        mask2 = const.tile([128, 2, KBLK], BF16)
        bqk_sb = const.tile([128, 2, 3], F32)

        wa = big.tile([128, CT, 3 * GC], BF16)
        wp = big.tile([128, NP, C], F32R)
        # K for heads 0-3 in fp8 d-half-pair layout: [32h+d%32, d//32, tok]
        kAB = big.tile([128, 2, T], F8)
        # K for heads 4,5: bf16 d-major
        kC = big.tile([128, T], BF16)
        # V interleaved with ones columns: even head h -> [V_h | 1],
        # odd head h -> [1 | V_h]; a single M=128 matmul then yields
        # y^T on one 64-partition half and the exp row-sums on the other.
        vs = big.tile([128, NK, HG, 2 * D], BF16)

        xt_r = xt_d[:, :].rearrange("(ct r) t -> ct r t", r=128)
        wa_r = wa_d[:, :].rearrange("(ct r) j -> ct r j", r=128)
        wp_r = wp_d[:, :].rearrange("(p r) e -> p r e", r=128)

        def dma_xtq(xtq, qs):
            for ct in range(CT):
                nc.sync.dma_start(out=xtq[:, ct, :],
                                  in_=xt_r[ct][:, qs:qs + QBLK])

        def qk_group(xtq, qAB, qC, qs, which, t):
            """One Q-or-K 128-channel psum tile: 6 matmuls + eviction
            (fp8 pair layout for tiles 0-1, bf16 d-major for tile 2),
            split into two half-closures so the interleaver can place
            them at sub-group granularity."""
            cell = {}

            def half(lo, hi, evict):
                def mms():
                    _mark(nc, f"qk.w{which}.t{t}")
                    if lo == 0:
                        cell["pqk"] = psB.tile([128, QBLK], F32, tag="b",
                                               name="pqk")
                    pqk = cell["pqk"]
                    for ct in range(lo, hi):
                        nc.tensor.matmul(
                            pqk,
                            lhsT=wa[:, ct, which * GC + t * 128:
                                           which * GC + (t + 1) * 128],
                            rhs=xtq[:, ct, :],
                            start=(ct == 0), stop=(ct == CT - 1))
                    if evict:
                        sc = bqk_sb[:, which, t:t + 1]
                        if t < 2:
                            dest = qAB[:, t, :] if which == 0 \
                                else kAB[:, t, qs:qs + QBLK]
                        else:
                            dest = qC if which == 0 \
                                else kC[:, qs:qs + QBLK]
                        nc.vector.tensor_scalar_add(dest, pqk, sc)
                return mms
            return [half(0, 3, False), half(3, CT, True)]

        def v_group(xtq, k_i, kl):
            def mms():
                _mark(nc, f"v.k{k_i}")
                pv = psB.tile([128, QBLK], F32, tag="b", name="pv")
                for ct in range(CT):
                    nc.tensor.matmul(
                        pv[:, 0:GC],
                        lhsT=xtq[:, ct, kl * KBLK:(kl + 1) * KBLK],
                        rhs=wa[:, ct, 2 * GC:3 * GC],
                        start=(ct == 0), stop=(ct == CT - 1))
                pv3 = pv[:, 0:GC].rearrange("r (a b d) -> r a b d", b=2, d=D)
                vsv = vs[:, k_i].rearrange("r (a b) c -> r a b c", b=2)
                vso = vs[:, k_i].rearrange("r (a b) c -> r a (b c)", b=2)
                # ones occupy the middle 128 cols of each pair's 256 block;
                # even head V -> cols 0:64, odd head V -> cols 64:128 of its
                # own block
                nc.gpsimd.memset(vso[:, :, D:3 * D], 1.0)
                nc.vector.tensor_copy(vsv[:, :, 0, 0:D], pv3[:, :, 0, :])
                nc.vector.tensor_copy(vsv[:, :, 1, D:2 * D], pv3[:, :, 1, :])
            return mms

        def proj_group(yt, qs, tt, split_dma=False):
            def mms():
                _mark(nc, f"proj.tt{tt}")
                t0 = qs + tt * KBLK
                ob = obp.tile([128, C], BF16, tag="ob", name="ob")
                for ec in range(2):
                    po = psB.tile([128, QBLK], F32, tag="b", name="po")
                    for j in range(NP):
                        nc.tensor.matmul(
                            po[:, 0:GC],
                            lhsT=yt[:, j, tt * KBLK:(tt + 1) * KBLK],
                            rhs=wp[:, j, ec * GC:(ec + 1) * GC],
                            start=(j == 0), stop=(j == NP - 1))
                    nc.vector.tensor_copy(ob[:, ec * GC:(ec + 1) * GC],
                                           po[:, 0:GC])
                    if split_dma:
                        # tail: fire each half as soon as it is evicted
                        q_eng = nc.sync if ec == 0 else nc.scalar
                        q_eng.dma_start(
                            out=out_d[t0:t0 + KBLK, ec * GC:(ec + 1) * GC],
                            in_=ob[:, ec * GC:(ec + 1) * GC])
                if not split_dma:
                    q_eng = nc.sync if tt % 2 == 0 else nc.scalar
                    q_eng.dma_start(out=out_d[t0:t0 + KBLK, :], in_=ob)
            return mms

        proj_queue = []         # deferred c_proj groups of earlier q-tiles
        carry = []              # fillers deferred to the next tile
        for q in range(NQ):
            qs = q * QBLK
            if q == 0:
                # startup: the QKV matmul for contraction tile ct needs the
                # (wa-qk[ct], xtq[ct]) pair, so stream those as interleaved
                # pairs on the two HWDGE queues; everything else follows.
                xtq = xtp.tile([128, CT, QBLK], BF16, tag="xtq", name="xtq")
                for ct in range(CT):
                    nc.scalar.dma_start(out=wa[:, ct, 0:2 * GC],
                                        in_=wa_r[ct][:, 0:2 * GC])
                    nc.gpsimd.dma_start(out=xtq[:, ct, :],
                                        in_=xt_r[ct][:, 0:QBLK])
                    if ct == 0:
                        nc.sync.dma_start(out=bqk_sb, in_=bqk_d[:, :, :])
                for ct in range(CT):
                    nc.scalar.dma_start(out=wa[:, ct, 2 * GC:3 * GC],
                                        in_=wa_r[ct][:, 2 * GC:3 * GC])
                pref_xtq = xtp.tile([128, CT, QBLK], BF16, tag="xtq",
                                    name="xtq")
                for ct in range(CT):
                    nc.gpsimd.dma_start(out=pref_xtq[:, ct, :],
                                        in_=xt_r[ct][:, QBLK:2 * QBLK])
                for pp in range(NP):
                    nc.scalar.dma_start(out=wp[:, pp, :], in_=wp_r[pp])
                make_upper_triangular(nc, mask2[:, 0, :], val=1.0, diag=True)
                make_upper_triangular(nc, mask2[:, 1, :], val=1.0, diag=True)
                qAB = qtp.tile([128, 2, QBLK], F8, tag="qAB", name="qAB")
                qC = qtp.tile([128, QBLK], BF16, tag="qC", name="qC")
                for which in (0, 1):
                    for t in range(3):
                        for h in qk_group(xtq, qAB, qC, qs, which, t):
                            h()
                for k_i in range(4):
                    v_group(xtq, k_i, k_i)()

            # fillers interleaved into this q-tile's attention stream.  The
            # early tiles are PE-rich (small attention) and the late ones
            # Act-bound, so PE-side work is skewed late: the last tile gets
            # the K-projection of its own QKV plus two tiles' worth of
            # c_proj from the deferred queue.
            fillers = list(carry)
            carry = []
            if q + 1 < NQ:
                nqs = qs + QBLK
                if q == 0:
                    nxtq = pref_xtq
                else:
                    nxtq = xtp.tile([128, CT, QBLK], BF16, tag="xtq",
                                    name="xtq")
                    dma_xtq(nxtq, nqs)
                nqAB = qtp.tile([128, 2, QBLK], F8, tag="qAB", name="qAB")
                nqC = qtp.tile([128, QBLK], BF16, tag="qC", name="qC")
                for t in range(3):
                    fillers += qk_group(nxtq, nqAB, nqC, nqs, 0, t)
                kq = [h for t in range(3)
                      for h in qk_group(nxtq, nqAB, nqC, nqs, 1, t)]
                if q + 1 == NQ - 1:
                    carry += kq      # K of the last tile: emit during it
                else:
                    fillers += kq
                for kl in range(4):
                    fillers.append(v_group(nxtq, 4 * (q + 1) + kl, kl))
            # deferred c_proj: none before att(2); proj(0) at att(2);
            # the rest at att(3)
            if q == NQ - 1:
                fillers += proj_queue
                proj_queue = []

            nkt = (q + 1) * (QBLK // KBLK)
            n_slots = NP * nkt
            yt = ytp.tile([128, NP, QBLK], F32R, tag="yt", name="yt")
            slot = 0
            emitted = 0
            for p in range(NP):
                ya = psY.tile([128, QBLK], F32, tag="y", name="ya")
                yb = psY.tile([128, QBLK], F32, tag="y", name="yb")
                def emit_S(k_i, p=p):
                    """S matmuls for (p, k_i); returns state for emit_exp."""
                    _mark(nc, f"att.q{q}.p{p}.k{k_i}")
                    m = k_i - 4 * q
                    col0 = max(m, 0) * KBLK
                    st2 = psS.tile([128, 2, QBLK], F32, tag="st", name="st2")
                    for s in range(2):
                        h = 2 * p + s
                        if h < 4:
                            base = 32 * h
                            nc.tensor.matmul(
                                st2[:, s, col0:QBLK],
                                lhsT=kAB[base:base + 32, :,
                                         k_i * KBLK:(k_i + 1) * KBLK],
                                rhs=qAB[base:base + 32, :, col0:QBLK],
                                start=True, stop=True, perf_mode=DR,
                                tile_position=(base, 0))
                        else:
                            hb = 64 * (h - 4)
                            nc.tensor.matmul(
                                st2[:, s, col0:QBLK],
                                lhsT=kC[hb:hb + 64,
                                        k_i * KBLK:(k_i + 1) * KBLK],
                                rhs=qC[hb:hb + 64, col0:QBLK],
                                start=True, stop=True)
                    return (k_i, m, col0, st2)

                # S runs one iteration ahead of exp/PV so a filler emitted
                # between iterations never delays the exp the Act engine is
                # about to run (engines execute their streams in order).
                pend = None
                sq = emit_S(0)
                for k_i in range(nkt):
                    if k_i + 1 < nkt:
                        nxt = emit_S(k_i + 1)
                    _, m, col0, st2 = sq
                    pt2 = ptp.tile([128, 2, QBLK], BF16, tag="pt", name="pt2")
                    nc.scalar.activation(pt2[:, :, col0:QBLK],
                                         st2[:, :, col0:QBLK],
                                         AF.Exp, scale=0.125)
                    if m >= 0:
                        seg = pt2[:, :, col0:col0 + KBLK]
                        nc.vector.tensor_mul(seg, seg, mask2)
                    if pend is not None:
                        pend()
                    first = (k_i == 0)
                    last = (k_i == nkt - 1)

                    def make_pv(pt2=pt2, p=p, col0=col0, first=first,
                                last=last, k_i=k_i, ya=ya, yb=yb):
                        def pv():
                            for s in range(2):
                                yy = ya if s == 0 else yb
                                nc.tensor.matmul(
                                    yy[:, col0:QBLK],
                                    lhsT=vs[:, k_i, 2 * p + s, :],
                                    rhs=pt2[:, s, col0:QBLK],
                                    start=first, stop=last,
                                    skip_group_check=True)
                        return pv
                    pend = make_pv()
                    if k_i + 1 < nkt:
                        sq = nxt
                    # interleave cross-phase matmul groups (front-loaded so
                    # dependency-critical groups land before their readers)
                    slot += 1
                    want = min(len(fillers),
                               (3 * slot * len(fillers)) // (2 * n_slots))
                    while emitted < want:
                        fillers[emitted]()
                        emitted += 1
                pend()
                _mark(nc, f"norm.q{q}.p{p}")
                # normalize: y^T / r.  Evict psum, partition-shift r via
                # gpsimd, single-pass reciprocal, two multiplies.  For the
                # final tile's last pair this is the tail critical path, so
                # run it in 128-column slices that unblock proj(tt) early.
                ya_sb = rp.tile([128, QBLK], F32, tag="ya", name="ya_sb")
                yb_sb = rp.tile([128, QBLK], F32, tag="yb", name="yb_sb")
                rsh = rp.tile([128, QBLK], F32, tag="rsh", name="rsh")
                rec = rp.tile([128, QBLK], F32, tag="rec", name="rec")
                tail = (q == NQ - 1 and p == NP - 1)
                n_sl = 4 if tail else 1
                for sl in range(n_sl):
                    cs = slice(sl * (QBLK // n_sl), (sl + 1) * (QBLK // n_sl))
                    nc.vector.tensor_copy(ya_sb[:, cs], ya[:, cs])
                    if tail:
                        # Act is idle once the last exp retires; use it for
                        # the second eviction to shorten the tail chain
                        nc.scalar.copy(yb_sb[:, cs], yb[:, cs])
                    else:
                        nc.vector.tensor_copy(yb_sb[:, cs], yb[:, cs])
                    nc.gpsimd.tensor_copy(rsh[0:64, cs], ya_sb[64:128, cs])
                    nc.gpsimd.tensor_copy(rsh[64:128, cs], yb_sb[0:64, cs])
                    nc.vector.reciprocal_approx_fast(rec[:, cs], rsh[:, cs])
                    nc.vector.tensor_mul(yt[0:64, p, cs], ya_sb[0:64, cs],
                                         rec[0:64, cs])
                    nc.vector.tensor_mul(yt[64:128, p, cs],
                                         yb_sb[64:128, cs],
                                         rec[64:128, cs])
            while emitted < len(fillers):
                fillers[emitted]()
                emitted += 1
            proj_queue += [proj_group(yt, qs, tt, split_dma=(q == NQ - 1))
                           for tt in range(QBLK // KBLK)]
            if q + 1 < NQ:
                xtq, qAB, qC = nxtq, nqAB, nqC
        for g in proj_queue:
            g()
    nc.compile()
    return nc


def make_in_map(x_b, w_attn, b_attn, w_proj, g):
    """Per-core input arrays for batch slice x_b and head-group g."""
    sl = slice(g * GC, (g + 1) * GC)
    perm = _qk_perm()
    wq = w_attn[:, 0 * C:1 * C][:, sl][:, perm]
    wk = w_attn[:, 1 * C:2 * C][:, sl][:, perm]
    wv = w_attn[:, 2 * C:3 * C][:, sl]
    bq = b_attn[0 * C:1 * C][sl][perm]
    bk = b_attn[1 * C:2 * C][sl][perm]
    # [128, 2, 3]: per-partition bias for the 3 Q/K psum tiles
    bqk = np.ascontiguousarray(
        np.stack([bq, bk]).reshape(2, 3, 128).transpose(2, 0, 1))
    import ml_dtypes
    return {
        "xt": np.ascontiguousarray(x_b.T).astype(ml_dtypes.bfloat16),
        "wa": np.ascontiguousarray(
            np.concatenate([wq, wk, wv], axis=1)).astype(ml_dtypes.bfloat16),
        "bqk": bqk,
        "wp": np.ascontiguousarray(w_proj[sl, :]),
    }


_NC_CACHE = {}


def _get_nc(T):
    if T not in _NC_CACHE:
        _NC_CACHE[T] = build_nc(T)
    return _NC_CACHE[T]


def kernel(x, w_attn, b_attn, w_proj, b_proj, _trace=False):
    from concourse.bass_utils import run_bass_kernel_spmd

    x = np.asarray(x, dtype=np.float32)
    w_attn = np.asarray(w_attn, dtype=np.float32)
    b_attn = np.asarray(b_attn, dtype=np.float32)
    w_proj = np.asarray(w_proj, dtype=np.float32)
    b_proj = np.asarray(b_proj, dtype=np.float32)
    B, T, _ = x.shape

    nc = _get_nc(T)
    in_maps = []
    for b in range(B):
        for g in range(2):
            in_maps.append(make_in_map(x[b], w_attn, b_attn, w_proj, g))
    res = run_bass_kernel_spmd(nc, in_maps, core_ids=list(range(2 * B)),
                               trace=_trace)
    outs = [np.asarray(r["out"], dtype=np.float32) for r in res.results]
    # softmax rows sum to 1, so the V-bias contribution is exactly
    # bv @ w_proj added to every token (not computed on device).
    bias_row = b_proj + b_attn[2 * C:3 * C] @ w_proj
    out = np.empty((B, T, C), dtype=np.float32)
    for b in range(B):
        out[b] = outs[2 * b] + outs[2 * b + 1] + bias_row[None, :]
    if _trace:
        kernel.last_result = res
    return out


# revision 64
# speedup vs baseline: 1.3412x; 1.0251x over previous
"""Causal self-attention (GPT-style, B=4 T=2048 C=768 H=12) on 8 trn2 cores.

Sharding: core = (batch b, head-group g), g in {0,1} covering 6 heads.
Each core: qkv projections for its 6 heads, causal flash-style attention,
partial c_proj over its 384 contraction rows; host adds the two partials
per batch plus the analytic bias row.

Key device-side structure (per core):
  x^T and w_attn ship as bf16 (halves DMA); output ships as bf16.
  QKV projections in bf16 via four 96-channel psum tiles per Q and K; a
  host-side w_attn column permutation puts 3 heads' d-halves per tile so
  Q/K evict to fp8e4 in a [32*(h%3), 2(d-half), T] pair layout with
  partition-identity copies.  All 6 heads then run S^T = K Q^T as fp8
  DoubleRow matmuls (half cycles/col; operand base partitions are
  restricted to {0,32,64}, hence 3 heads per 96-partition tile pair).
  exp on ScalarE over both heads of a pair at once ([128, 2, w] from a
  2-bank psum tile) -> P in bf16; causal diagonal masked post-exp by a
  DVE multiply.
  [V_h | 1] interleaved bf16 matmul accumulates y^T (64 partitions) and
  softmax row-sums (other 64) per (head, k-tile) into one psum bank; the
  ones blocks are memset per k-tile, V evicted with strided copies.
  Normalize: evict psum, gpsimd partition-shift of the row-sums, one
  fast reciprocal, two multiplies -> y^T fp32 (column-sliced for the
  final pair so the tail c_proj starts early).
  proj: out[t,e] = sum_f y^T[f,t] wp[f,e] in fp32r.

Scheduling: engines execute their streams in order, so PV(k-1) is issued
after S(k) -- the PE stream never blocks on exp(k) -- and cross-phase
matmul groups (next tile's QKV/V, deferred c_proj) are interleaved as
fillers into the attention loop, skewed toward the later (Act-bound)
tiles.  Startup streams (wa[ct], x[ct]) pairs on separate DGE queues so
the first QKV matmul fires as early as possible.

fp8 S gives ~1.16e-2 end-to-end rel err (vs 2.2e-3 all-bf16; gate 2e-2).
fp8 for QKV inputs, P, or V was measured at 3.7e-2..4.6e-2 -- rejected.
"""

from contextlib import ExitStack

import numpy as np

import concourse.bass as bass
import concourse.mybir as mybir
import concourse.tile as tile
from concourse import bacc
from concourse.masks import make_upper_triangular

AF = mybir.ActivationFunctionType
F32 = mybir.dt.float32
F32R = mybir.dt.float32r
BF16 = mybir.dt.bfloat16
F8 = mybir.dt.float8e4
DR = mybir.MatmulPerfMode.DoubleRow

C = 768          # model dim
D = 64           # head dim
HG = 6           # heads per core
NP = 3           # head pairs per core
GC = HG * D      # 384 group channels
CT = C // 128    # 6 contraction tiles
QBLK = 512       # query tile (psum bank)
KBLK = 128       # key tile (partition dim)

# All 6 heads run the S matmul as fp8e4 DoubleRow (half cycles/col).
# ~1.1e-2 end-to-end rel err (vs 2.2e-3 all-bf16); gate is 2e-2.
# Matmul operand base partitions are limited to {0,32,64}, so the Q/K
# channels are grouped 3 heads per 96-channel psum tile: tile t holds
# d-half (t%2) of heads 3*(t//2)..3*(t//2)+2.


def _qk_perm():
    """Channel permutation (within the 384 group channels) for Q and K.
    Tile 0 holds d0-31 of heads 0-3, tile 1 d32-63 of heads 0-3, tile 2
    heads 4-5 in natural d-major order.  perm[n] = original channel
    feeding new channel n."""
    perm = np.empty(GC, dtype=np.int64)
    for n in range(GC):
        t, slot = divmod(n, 128)
        if t < 2:
            head, dd = divmod(slot, 32)
            perm[n] = head * D + 32 * t + dd
        else:
            head, dd = divmod(slot, 64)
            perm[n] = (4 + head) * D + dd
    return perm


_REGIONS = []      # (label, next_instruction_index) probes for trace analysis


def _mark(nc, label):
    _REGIONS.append((label,
                     int(nc.get_next_instruction_name().split("-")[-1])))


def build_nc(T=2048):
    NQ = T // QBLK
    NK = T // KBLK
    nc = bacc.Bacc(None)

    xt_d = nc.dram_tensor("xt", [C, T], BF16, kind="ExternalInput")
    wa_d = nc.dram_tensor("wa", [C, 3 * GC], BF16, kind="ExternalInput")
    bqk_d = nc.dram_tensor("bqk", [128, 2, 3], F32, kind="ExternalInput")
    wp_d = nc.dram_tensor("wp", [GC, C], F32R, kind="ExternalInput")
    out_d = nc.dram_tensor("out", [T, C], BF16, kind="ExternalOutput")

    with ExitStack() as ctx:
        tc = ctx.enter_context(tile.TileContext(nc))
        const = ctx.enter_context(tc.tile_pool(name="const", bufs=1))
        big = ctx.enter_context(tc.tile_pool(name="big", bufs=1))
        xtp = ctx.enter_context(tc.tile_pool(name="xtp", bufs=2))
        qtp = ctx.enter_context(tc.tile_pool(name="qtp", bufs=2))
        ytp = ctx.enter_context(tc.tile_pool(name="ytp", bufs=3))
        ptp = ctx.enter_context(tc.tile_pool(name="ptp", bufs=6))
        rp = ctx.enter_context(tc.tile_pool(name="rp", bufs=2))
        obp = ctx.enter_context(tc.tile_pool(name="obp", bufs=2))
        psS = ctx.enter_context(tc.tile_pool(name="psS", bufs=2, space="PSUM"))
        psY = ctx.enter_context(tc.tile_pool(name="psY", bufs=2, space="PSUM"))
        psB = ctx.enter_context(tc.tile_pool(name="psB", bufs=2, space="PSUM"))

        # causal mask, replicated for the two heads of an exp pair
        # (built after the startup DMAs are issued -- see q == 0 below --
        # so the gpsimd queue isn't busy ahead of the SWDGE x fetch)
        mask2 = const.tile([128, 2, KBLK], BF16)
        bqk_sb = const.tile([128, 2, 3], F32)

        wa = big.tile([128, CT, 3 * GC], BF16)
        wp = big.tile([128, NP, C], F32R)
        # K for heads 0-3 in fp8 d-half-pair layout: [32h+d%32, d//32, tok]
        kAB = big.tile([128, 2, T], F8)
        # K for heads 4,5: bf16 d-major
        kC = big.tile([128, T], BF16)
        # V interleaved with ones columns: even head h -> [V_h | 1],
        # odd head h -> [1 | V_h]; a single M=128 matmul then yields
        # y^T on one 64-partition half and the exp row-sums on the other.
        vs = big.tile([128, NK, HG, 2 * D], BF16)

        xt_r = xt_d[:, :].rearrange("(ct r) t -> ct r t", r=128)
        wa_r = wa_d[:, :].rearrange("(ct r) j -> ct r j", r=128)
        wp_r = wp_d[:, :].rearrange("(p r) e -> p r e", r=128)

        def dma_xtq(xtq, qs):
            for ct in range(CT):
                nc.sync.dma_start(out=xtq[:, ct, :],
                                  in_=xt_r[ct][:, qs:qs + QBLK])

        def qk_group(xtq, qAB, qC, qs, which, t):
            """One Q-or-K 128-channel psum tile: 6 matmuls + eviction
            (fp8 pair layout for tiles 0-1, bf16 d-major for tile 2),
            split into two half-closures so the interleaver can place
            them at sub-group granularity."""
            cell = {}

            def half(lo, hi, evict):
                def mms():
                    _mark(nc, f"qk.w{which}.t{t}")
                    if lo == 0:
                        cell["pqk"] = psB.tile([128, QBLK], F32, tag="b",
                                               name="pqk")
                    pqk = cell["pqk"]
                    for ct in range(lo, hi):
                        nc.tensor.matmul(
                            pqk,
                            lhsT=wa[:, ct, which * GC + t * 128:
                                           which * GC + (t + 1) * 128],
                            rhs=xtq[:, ct, :],
                            start=(ct == 0), stop=(ct == CT - 1))
                    if evict:
                        sc = bqk_sb[:, which, t:t + 1]
                        if t < 2:
                            dest = qAB[:, t, :] if which == 0 \
                                else kAB[:, t, qs:qs + QBLK]
                        else:
                            dest = qC if which == 0 \
                                else kC[:, qs:qs + QBLK]
                        nc.vector.tensor_scalar_add(dest, pqk, sc)
                return mms
            return [half(0, 3, False), half(3, CT, True)]

        def v_group(xtq, k_i, kl):
            def mms():
                _mark(nc, f"v.k{k_i}")
                pv = psB.tile([128, QBLK], F32, tag="b", name="pv")
                for ct in range(CT):
                    nc.tensor.matmul(
                        pv[:, 0:GC],
                        lhsT=xtq[:, ct, kl * KBLK:(kl + 1) * KBLK],
                        rhs=wa[:, ct, 2 * GC:3 * GC],
                        start=(ct == 0), stop=(ct == CT - 1))
                pv3 = pv[:, 0:GC].rearrange("r (a b d) -> r a b d", b=2, d=D)
                vsv = vs[:, k_i].rearrange("r (a b) c -> r a b c", b=2)
                vso = vs[:, k_i].rearrange("r (a b) c -> r a (b c)", b=2)
                # ones occupy the middle 128 cols of each pair's 256 block;
                # even head V -> cols 0:64, odd head V -> cols 64:128 of its
                # own block
                nc.gpsimd.memset(vso[:, :, D:3 * D], 1.0)
                nc.vector.tensor_copy(vsv[:, :, 0, 0:D], pv3[:, :, 0, :])
                nc.vector.tensor_copy(vsv[:, :, 1, D:2 * D], pv3[:, :, 1, :])
            return mms

        def proj_group(yt, qs, tt, split_dma=False):
            def mms():
                _mark(nc, f"proj.tt{tt}")
                t0 = qs + tt * KBLK
                ob = obp.tile([128, C], BF16, tag="ob", name="ob")
                for ec in range(2):
                    po = psB.tile([128, QBLK], F32, tag="b", name="po")
                    for j in range(NP):
                        nc.tensor.matmul(
                            po[:, 0:GC],
                            lhsT=yt[:, j, tt * KBLK:(tt + 1) * KBLK],
                            rhs=wp[:, j, ec * GC:(ec + 1) * GC],
                            start=(j == 0), stop=(j == NP - 1))
                    if split_dma and ec == 1:
                        # tail: Act is idle; do the second eviction there so
                        # both halves evict in parallel
                        nc.scalar.copy(ob[:, ec * GC:(ec + 1) * GC],
                                       po[:, 0:GC])
                    else:
                        nc.vector.tensor_copy(ob[:, ec * GC:(ec + 1) * GC],
                                              po[:, 0:GC])
                    if split_dma:
                        # tail: fire each half as soon as it is evicted
                        q_eng = nc.sync if ec == 0 else nc.scalar
                        q_eng.dma_start(
                            out=out_d[t0:t0 + KBLK, ec * GC:(ec + 1) * GC],
                            in_=ob[:, ec * GC:(ec + 1) * GC])
                if not split_dma:
                    q_eng = nc.sync if tt % 2 == 0 else nc.scalar
                    q_eng.dma_start(out=out_d[t0:t0 + KBLK, :], in_=ob)
            return mms

        proj_queue = []         # deferred c_proj groups of earlier q-tiles
        carry = []              # fillers deferred to the next tile
        for q in range(NQ):
            qs = q * QBLK
            if q == 0:
                # startup: the QKV matmul for contraction tile ct needs the
                # (wa-qk[ct], xtq[ct]) pair, so stream those as interleaved
                # pairs on the two HWDGE queues; everything else follows.
                xtq = xtp.tile([128, CT, QBLK], BF16, tag="xtq", name="xtq")
                for ct in range(CT):
                    nc.scalar.dma_start(out=wa[:, ct, 0:2 * GC],
                                        in_=wa_r[ct][:, 0:2 * GC])
                    nc.gpsimd.dma_start(out=xtq[:, ct, :],
                                        in_=xt_r[ct][:, 0:QBLK])
                    if ct == 0:
                        nc.sync.dma_start(out=bqk_sb, in_=bqk_d[:, :, :])
                for ct in range(CT):
                    nc.scalar.dma_start(out=wa[:, ct, 2 * GC:3 * GC],
                                        in_=wa_r[ct][:, 2 * GC:3 * GC])
                pref_xtq = xtp.tile([128, CT, QBLK], BF16, tag="xtq",
                                    name="xtq")
                for ct in range(CT):
                    nc.gpsimd.dma_start(out=pref_xtq[:, ct, :],
                                        in_=xt_r[ct][:, QBLK:2 * QBLK])
                for pp in range(NP):
                    nc.scalar.dma_start(out=wp[:, pp, :], in_=wp_r[pp])
                make_upper_triangular(nc, mask2[:, 0, :], val=1.0, diag=True)
                make_upper_triangular(nc, mask2[:, 1, :], val=1.0, diag=True)
                qAB = qtp.tile([128, 2, QBLK], F8, tag="qAB", name="qAB")
                qC = qtp.tile([128, QBLK], BF16, tag="qC", name="qC")
                for which in (0, 1):
                    for t in range(3):
                        for h in qk_group(xtq, qAB, qC, qs, which, t):
                            h()
                for k_i in range(4):
                    v_group(xtq, k_i, k_i)()

            # fillers interleaved into this q-tile's attention stream.  The
            # early tiles are PE-rich (small attention) and the late ones
            # Act-bound, so PE-side work is skewed late: the last tile gets
            # the K-projection of its own QKV plus two tiles' worth of
            # c_proj from the deferred queue.
            fillers = list(carry)
            carry = []
            if q + 1 < NQ:
                nqs = qs + QBLK
                if q == 0:
                    nxtq = pref_xtq
                else:
                    nxtq = xtp.tile([128, CT, QBLK], BF16, tag="xtq",
                                    name="xtq")
                    dma_xtq(nxtq, nqs)
                nqAB = qtp.tile([128, 2, QBLK], F8, tag="qAB", name="qAB")
                nqC = qtp.tile([128, QBLK], BF16, tag="qC", name="qC")
                for t in range(3):
                    fillers += qk_group(nxtq, nqAB, nqC, nqs, 0, t)
                kq = [h for t in range(3)
                      for h in qk_group(nxtq, nqAB, nqC, nqs, 1, t)]
                if q + 1 == NQ - 1:
                    carry += kq      # K of the last tile: emit during it
                else:
                    fillers += kq
                for kl in range(4):
                    fillers.append(v_group(nxtq, 4 * (q + 1) + kl, kl))
            # deferred c_proj: none before att(2); proj(0) at att(2);
            # the rest at att(3)
            if q == NQ - 1:
                fillers += proj_queue
                proj_queue = []

            nkt = (q + 1) * (QBLK // KBLK)
            n_slots = NP * nkt
            yt = ytp.tile([128, NP, QBLK], F32R, tag="yt", name="yt")
            slot = 0
            emitted = 0
            for p in range(NP):
                ya = psY.tile([128, QBLK], F32, tag="y", name="ya")
                yb = psY.tile([128, QBLK], F32, tag="y", name="yb")
                def emit_S(k_i, p=p):
                    """S matmuls for (p, k_i); returns state for emit_exp."""
                    _mark(nc, f"att.q{q}.p{p}.k{k_i}")
                    m = k_i - 4 * q
                    col0 = max(m, 0) * KBLK
                    st2 = psS.tile([128, 2, QBLK], F32, tag="st", name="st2")
                    for s in range(2):
                        h = 2 * p + s
                        if h < 4:
                            base = 32 * h
                            nc.tensor.matmul(
                                st2[:, s, col0:QBLK],
                                lhsT=kAB[base:base + 32, :,
                                         k_i * KBLK:(k_i + 1) * KBLK],
                                rhs=qAB[base:base + 32, :, col0:QBLK],
                                start=True, stop=True, perf_mode=DR,
                                tile_position=(base, 0))
                        else:
                            hb = 64 * (h - 4)
                            nc.tensor.matmul(
                                st2[:, s, col0:QBLK],
                                lhsT=kC[hb:hb + 64,
                                        k_i * KBLK:(k_i + 1) * KBLK],
                                rhs=qC[hb:hb + 64, col0:QBLK],
                                start=True, stop=True)
                    return (k_i, m, col0, st2)

                # S runs one iteration ahead of exp/PV so a filler emitted
                # between iterations never delays the exp the Act engine is
                # about to run (engines execute their streams in order).
                pend = None
                sq = emit_S(0)
                for k_i in range(nkt):
                    if k_i + 1 < nkt:
                        nxt = emit_S(k_i + 1)
                    _, m, col0, st2 = sq
                    pt2 = ptp.tile([128, 2, QBLK], BF16, tag="pt", name="pt2")
                    nc.scalar.activation(pt2[:, :, col0:QBLK],
                                         st2[:, :, col0:QBLK],
                                         AF.Exp, scale=0.125)
                    if m >= 0:
                        seg = pt2[:, :, col0:col0 + KBLK]
                        nc.vector.tensor_mul(seg, seg, mask2)
                    if pend is not None:
                        pend()
                    first = (k_i == 0)
                    last = (k_i == nkt - 1)

                    def make_pv(pt2=pt2, p=p, col0=col0, first=first,
                                last=last, k_i=k_i, ya=ya, yb=yb):
                        def pv():
                            for s in range(2):
                                yy = ya if s == 0 else yb
                                nc.tensor.matmul(
                                    yy[:, col0:QBLK],
                                    lhsT=vs[:, k_i, 2 * p + s, :],
                                    rhs=pt2[:, s, col0:QBLK],
                                    start=first, stop=last,
                                    skip_group_check=True)
                        return pv
                    pend = make_pv()
                    if k_i + 1 < nkt:
                        sq = nxt
                    # interleave cross-phase matmul groups (front-loaded so
                    # dependency-critical groups land before their readers)
                    slot += 1
                    want = min(len(fillers),
                               (slot * len(fillers)) // n_slots)
                    while emitted < want:
                        fillers[emitted]()
                        emitted += 1
                pend()
                _mark(nc, f"norm.q{q}.p{p}")
                # normalize: y^T / r.  Evict psum, partition-shift r via
                # gpsimd, single-pass reciprocal, two multiplies.  For the
                # final tile's last pair this is the tail critical path, so
                # run it in 128-column slices that unblock proj(tt) early.
                ya_sb = rp.tile([128, QBLK], F32, tag="ya", name="ya_sb")
                yb_sb = rp.tile([128, QBLK], F32, tag="yb", name="yb_sb")
                rsh = rp.tile([128, QBLK], F32, tag="rsh", name="rsh")
                rec = rp.tile([128, QBLK], F32, tag="rec", name="rec")
                tail = (q == NQ - 1 and p == NP - 1)
                n_sl = 4 if tail else 1
                for sl in range(n_sl):
                    cs = slice(sl * (QBLK // n_sl), (sl + 1) * (QBLK // n_sl))
                    nc.vector.tensor_copy(ya_sb[:, cs], ya[:, cs])
                    if tail:
                        # Act is idle once the last exp retires; use it for
                        # the second eviction to shorten the tail chain
                        nc.scalar.copy(yb_sb[:, cs], yb[:, cs])
                    else:
                        nc.vector.tensor_copy(yb_sb[:, cs], yb[:, cs])
                    nc.gpsimd.tensor_copy(rsh[0:64, cs], ya_sb[64:128, cs])
                    nc.gpsimd.tensor_copy(rsh[64:128, cs], yb_sb[0:64, cs])
                    nc.vector.reciprocal_approx_fast(rec[:, cs], rsh[:, cs])
                    nc.vector.tensor_mul(yt[0:64, p, cs], ya_sb[0:64, cs],
                                         rec[0:64, cs])
                    nc.vector.tensor_mul(yt[64:128, p, cs],
                                         yb_sb[64:128, cs],
                                         rec[64:128, cs])
            while emitted < len(fillers):
                fillers[emitted]()
                emitted += 1
            proj_queue += [proj_group(yt, qs, tt, split_dma=(q == NQ - 1))
                           for tt in range(QBLK // KBLK)]
            if q + 1 < NQ:
                xtq, qAB, qC = nxtq, nqAB, nqC
        for g in proj_queue:
            g()
    nc.compile()
    return nc


def make_in_map(x_b, w_attn, b_attn, w_proj, g):
    """Per-core input arrays for batch slice x_b and head-group g."""
    sl = slice(g * GC, (g + 1) * GC)
    perm = _qk_perm()
    wq = w_attn[:, 0 * C:1 * C][:, sl][:, perm]
    wk = w_attn[:, 1 * C:2 * C][:, sl][:, perm]
    wv = w_attn[:, 2 * C:3 * C][:, sl]
    bq = b_attn[0 * C:1 * C][sl][perm]
    bk = b_attn[1 * C:2 * C][sl][perm]
    # [128, 2, 3]: per-partition bias for the 3 Q/K psum tiles
    bqk = np.ascontiguousarray(
        np.stack([bq, bk]).reshape(2, 3, 128).transpose(2, 0, 1))
    import ml_dtypes
    return {
        "xt": np.ascontiguousarray(x_b.T).astype(ml_dtypes.bfloat16),
        "wa": np.ascontiguousarray(
            np.concatenate([wq, wk, wv], axis=1)).astype(ml_dtypes.bfloat16),
        "bqk": bqk,
        "wp": np.ascontiguousarray(w_proj[sl, :]),
    }


_NC_CACHE = {}


def _get_nc(T):
    if T not in _NC_CACHE:
        _NC_CACHE[T] = build_nc(T)
    return _NC_CACHE[T]


def kernel(x, w_attn, b_attn, w_proj, b_proj, _trace=False):
    from concourse.bass_utils import run_bass_kernel_spmd

    x = np.asarray(x, dtype=np.float32)
    w_attn = np.asarray(w_attn, dtype=np.float32)
    b_attn = np.asarray(b_attn, dtype=np.float32)
    w_proj = np.asarray(w_proj, dtype=np.float32)
    b_proj = np.asarray(b_proj, dtype=np.float32)
    B, T, _ = x.shape

    nc = _get_nc(T)
    in_maps = []
    for b in range(B):
        for g in range(2):
            in_maps.append(make_in_map(x[b], w_attn, b_attn, w_proj, g))
    res = run_bass_kernel_spmd(nc, in_maps, core_ids=list(range(2 * B)),
                               trace=_trace)
    outs = [np.asarray(r["out"], dtype=np.float32) for r in res.results]
    # softmax rows sum to 1, so the V-bias contribution is exactly
    # bv @ w_proj added to every token (not computed on device).
    bias_row = b_proj + b_attn[2 * C:3 * C] @ w_proj
    out = np.empty((B, T, C), dtype=np.float32)
    for b in range(B):
        out[b] = outs[2 * b] + outs[2 * b + 1] + bias_row[None, :]
    if _trace:
        kernel.last_result = res
    return out
